# revision 7
# baseline (speedup 1.0000x reference)
"""Trainium2 Bass kernel for nn_CrossModalAttention (B=2, LQ=LK=2048,
QDIM=HID=1024, KDIM=VDIM=768, H=16, D=64).

Sharding: 8 cores = 2 batches x 4 head-groups (4 heads each).
Per core: q/k/v projections column-sliced over HID, attention for its 4
heads, row-parallel partial of the out-projection. Host sums the 4
partials per batch (the row-parallel unshard) and adds bo.

Device dataflow (per core), all matmuls in fp32r (TF32-like, ~1.5e-4):
  - host passes query/key/value[b] transposed (and K/V key-compacted:
    query_mask masks the KEY axis globally per batch, so masked keys are
    dropped on host and the remainder padded to a multiple of 128)
  - qT/kT [hid, tokens] and v [keys, hid] computed on device
  - per head pair (row-packed K=64 matmuls via tile_position):
    scoresT [keys, q] -> ACT exp(s/8 + mask_bias) -> PV matmul with a
    ones-augmented V (M=65) giving ctxT and the softmax denominator
  - normalize on DVE (reciprocal + gpsimd partition-broadcast)
  - out-projection from ctxT, partial written to DRAM
"""

import math

import ml_dtypes
import numpy as np

B, LQ, LK = 2, 2048, 2048
QDIM, KDIM, VDIM, HID, H = 1024, 768, 768, 1024, 16
D = HID // H  # 64
HG = 4  # head-groups (cores per batch)
HL = H // HG  # heads per core = 4
GH = HL * D  # per-core hid slice = 256
N_CORES = 8
TB = 512  # token block
NTB = LQ // TB  # 4
NEG = -1.0e30

BF16 = True
PROFILE = False
LAST_EXEC_NS = None
LAST_TRACE_DIR = None

_CACHE = {}


def _build(nkt: int, with_bv: bool, bf16: bool):
    import concourse.bacc as bacc
    import concourse.mybir as mybir
    import concourse.tile as tile

    nkeys = nkt * 128
    # key blocks of <=512 for the k-projection
    kbs = [min(512, nkeys - s) for s in range(0, nkeys, 512)]

    f32 = mybir.dt.float32
    f32r = mybir.dt.bfloat16 if bf16 else mybir.dt.float32r
    Exp = mybir.ActivationFunctionType.Exp
    Ident = mybir.ActivationFunctionType.Identity

    nc = bacc.Bacc(
        "TRN2", target_bir_lowering=False, debug=False, num_devices=N_CORES
    )

    # DRAM tensors (per-core shapes)
    XQ = nc.dram_tensor("xq", [128, 8, LQ], f32r, kind="ExternalInput").ap()
    XK = nc.dram_tensor("xk", [128, 6, nkeys], f32r, kind="ExternalInput").ap()
    XV = nc.dram_tensor("xv", [128, 6, nkeys], f32r, kind="ExternalInput").ap()
    WQ = nc.dram_tensor("wq", [128, 8, GH], f32r, kind="ExternalInput").ap()
    WK = nc.dram_tensor("wk", [128, 6, GH], f32r, kind="ExternalInput").ap()
    WV = nc.dram_tensor("wv", [128, 6, GH], f32r, kind="ExternalInput").ap()
    WO = nc.dram_tensor("wo", [128, 2, QDIM], f32r, kind="ExternalInput").ap()
    MB = nc.dram_tensor("mbias", [128, nkt], f32, kind="ExternalInput").ap()
    BQ = nc.dram_tensor("bqk", [128, 4], f32, kind="ExternalInput").ap()
    BV = None
    if with_bv:
        BV = nc.dram_tensor("bv", [128, 2], f32, kind="ExternalInput").ap()
    OUT = nc.dram_tensor("outp", [LQ, QDIM], f32, kind="ExternalOutput").ap()

    with tile.TileContext(nc) as tc:
        with (
            tc.tile_pool(name="consts", bufs=1) as consts,
            tc.tile_pool(name="resid", bufs=1) as resid,
            tc.tile_pool(name="xs", bufs=2) as xs,
            tc.tile_pool(name="probs", bufs=4) as probs_pool,
            tc.tile_pool(name="norm", bufs=3) as norm_pool,
            tc.tile_pool(name="outs", bufs=3) as outs_pool,
            tc.tile_pool(name="ps", bufs=2, space="PSUM") as ps,
        ):
            # ---- constants / weights ----
            # weights go on the gpsimd SWDGE ring so the big input streams
            # (sync HWDGE ring) aren't serialized behind them
            wq_sb = consts.tile([128, 8, GH], f32r)
            wk_sb = consts.tile([128, 6, GH], f32r)
            wv_sb = consts.tile([128, 6, GH], f32r)
            wo_sb = consts.tile([128, 2, QDIM], f32r)
            mb_sb = consts.tile([128, nkt], f32)
            bqk_sb = consts.tile([128, 4], f32)
            nc.gpsimd.dma_start(out=wk_sb, in_=WK)
            nc.scalar.dma_start(out=bqk_sb, in_=BQ)
            nc.scalar.dma_start(out=mb_sb, in_=MB)
            nc.gpsimd.dma_start(out=wv_sb, in_=WV)
            nc.gpsimd.dma_start(out=wq_sb, in_=WQ)
            nc.gpsimd.dma_start(out=wo_sb, in_=WO)
            bv_sb = None
            if with_bv:
                bv_sb = consts.tile([128, 2], f32)
                nc.gpsimd.dma_start(out=bv_sb, in_=BV)

            # ---- residents ----
            # qT tiles double as ctxT tiles later (WAR handled by Tile)
            qT = [resid.tile([128, LQ], f32r, tag=f"qT{p}", name=f"qT{p}") for p in range(2)]
            kT = [resid.tile([128, nkeys], f32r, tag=f"kT{p}", name=f"kT{p}") for p in range(2)]
            v_sb = resid.tile([128, nkt, HL, D + 1], f32r)
            # ones columns for the denominator rows: fill the whole tile,
            # the v-projection copies then overwrite the [., ., ., 0:D] part
            if bf16:
                nc.vector.memset(v_sb, 1.0)
            else:
                nc.vector.memset(v_sb[:, :, :, :].bitcast(f32), 1.0)

            # ---- k projection (per key-block, so attention can begin
            # after kb0) ----
            def emit_xk(kb_i):
                kbw = kbs[kb_i]
                s0 = kb_i * 512
                xk_t = xs.tile([128, 6, 512], f32r, tag="xk", name="xk_t", bufs=3)
                nc.sync.dma_start(
                    out=xk_t[:, :, :kbw], in_=XK[:, :, s0 : s0 + kbw]
                )
                return xk_t

            def emit_kproj_m(kb_i, m, xk_t):
                kbw = kbs[kb_i]
                s0 = kb_i * 512
                ps_t = ps.tile([128, 512], f32, tag="proj", name="kp_ps")
                for k in range(6):
                    nc.tensor.matmul(
                        ps_t[:, :kbw],
                        wk_sb[:, k, m * 128 : (m + 1) * 128],
                        xk_t[:, k, :kbw],
                        start=(k == 0),
                        stop=(k == 5),
                    )
                nc.vector.tensor_scalar_add(
                    kT[m][:, s0 : s0 + kbw],
                    ps_t[:, :kbw],
                    bqk_sb[:, 2 + m : 3 + m],
                )

            # ---- v projection (emitted later for tb0/p0 split; see
            # emit_vproj) ----
            vproj_state = {}

            def emit_vproj_kt(kt):
                kb_i = kt // 4
                sub = kt % 4
                if sub == 0 and kb_i not in vproj_state:
                    kbw = kbs[kb_i]
                    xv_t = xs.tile(
                        [128, 6, 512], f32r, tag="xv", name="xv_t"
                    )
                    nc.sync.dma_start(
                        out=xv_t[:, :, :kbw],
                        in_=XV[:, :, kb_i * 512 : kb_i * 512 + kbw],
                    )
                    vproj_state[kb_i] = xv_t
                xv_t = vproj_state[kb_i]
                ps_t = ps.tile([128, 512], f32, tag="proj", name="vp_ps")
                for k in range(6):
                    nc.tensor.matmul(
                        ps_t[:, :GH],
                        xv_t[:, k, sub * 128 : (sub + 1) * 128],
                        wv_sb[:, k, :],
                        start=(k == 0),
                        stop=(k == 5),
                    )
                nc.vector.tensor_copy(
                    v_sb[:, kt, :, 0:D],
                    ps_t[:, :GH].rearrange("p (h d) -> p h d", h=HL),
                )

            def emit_xq(tb):
                t0 = tb * TB
                xq_t = xs.tile([128, 8, TB], f32r, tag="xq", name="xq_t")
                nc.sync.dma_start(out=xq_t, in_=XQ[:, :, t0 : t0 + TB])
                return xq_t

            def emit_qproj_m(tb, m, xq_t):
                t0 = tb * TB
                ps_t = ps.tile([128, 512], f32, tag="proj", name="qp_ps")
                for k in range(8):
                    nc.tensor.matmul(
                        ps_t,
                        wq_sb[:, k, m * 128 : (m + 1) * 128],
                        xq_t[:, k, :],
                        start=(k == 0),
                        stop=(k == 7),
                    )
                nc.vector.tensor_scalar_add(
                    qT[m][:, t0 : t0 + TB],
                    ps_t,
                    bqk_sb[:, m : m + 1],
                )

            def emit_qproj(tb):
                xq_t = emit_xq(tb)
                for m in range(2):
                    emit_qproj_m(tb, m, xq_t)

            def emit_scores(p, tb, kt, prtag, prbufs):
                t0 = tb * TB
                k0 = kt * 128
                sc = ps.tile([128, 2, TB], f32, tag="sc", name="sc")
                for hh in range(2):
                    nc.tensor.matmul(
                        sc[:, hh, :],
                        kT[p][hh * 64 : hh * 64 + 64, k0 : k0 + 128],
                        qT[p][hh * 64 : hh * 64 + 64, t0 : t0 + TB],
                        start=True,
                        stop=True,
                        tile_position=(hh * 64, 0),
                    )
                pr = probs_pool.tile(
                    [128, 2, TB], f32r, tag=prtag, name="pr", bufs=prbufs
                )
                nc.scalar.activation(
                    pr, sc, Exp, bias=mb_sb[:, kt : kt + 1], scale=0.125
                )
                return pr

            def emit_pv(p, tb, kt, pr, ctx_ps):
                for hh in range(2):
                    nc.tensor.matmul(
                        ctx_ps[hh],
                        v_sb[:, kt, 2 * p + hh, :],
                        pr[:, hh, :],
                        start=(kt == 0),
                        stop=(kt == nkt - 1),
                    )

            def emit_normalize(p, tb, ctx_ps):
                t0 = tb * TB
                # evacuate both PSUM ctx tiles to SBUF first (quick DVE
                # copies) so the next block's PV matmuls get their PSUM
                # slots immediately; the normalize chain then runs from
                # SBUF off the PE critical path
                evac = []
                for hh in range(2):
                    ctmp = norm_pool.tile([D, TB], f32, tag="ctmp", name="ctmp")
                    nc.vector.tensor_copy(ctmp, ctx_ps[hh][0:D, :])
                    dcp = norm_pool.tile([1, TB], f32, tag="dcp", name="dcp")
                    nc.vector.tensor_copy(dcp, ctx_ps[hh][D : D + 1, :])
                    evac.append((ctmp, dcp))
                for hh in range(2):
                    ctmp, dcp = evac[hh]
                    rec1 = norm_pool.tile([1, TB], f32, tag="rec1", name="rec1")
                    nc.vector.reciprocal_approx_fast(out=rec1, in_=dcp)
                    rbc = norm_pool.tile([D, TB], f32, tag="rbc", name="rbc")
                    nc.gpsimd.partition_broadcast(rbc, rec1)
                    dst = qT[p][hh * 64 : hh * 64 + 64, t0 : t0 + TB]
                    nc.vector.tensor_mul(dst, ctmp, rbc)
                    if with_bv:
                        nc.vector.tensor_scalar_add(
                            dst, dst, bv_sb[64 * hh : 64 * hh + 64, p : p + 1]
                        )

            def emit_attn(p, tb, thunks=()):
                # thunk runs between the scores pair and the PV pair of each
                # kt: the interleaved PE work hides the exp(kt) latency the
                # PV would otherwise stall on
                thunks = list(thunks)
                ctx_ps = [
                    ps.tile([D + 1, TB], f32, tag="ctx", name=f"ctx{p}_{tb}_{i}")
                    for i in range(2)
                ]
                for kt in range(nkt):
                    pr = emit_scores(p, tb, kt, "pr", 4)
                    if kt < len(thunks):
                        thunks[kt]()
                    emit_pv(p, tb, kt, pr, ctx_ps)
                for th in thunks[nkt:]:
                    th()
                emit_normalize(p, tb, ctx_ps)

            def qproj_thunks(tb):
                # 6 small thunks: 3+3+2 matmuls per m-half, chained into
                # one psum accumulation (interleaving with other banks is
                # legal), evac on the last
                cell = {}

                def start(tb=tb):
                    cell["xq"] = emit_xq(tb)

                def chunk(m, k0, k1, fin, tb=tb):
                    if k0 == 0:
                        cell[m] = ps.tile(
                            [128, 512], f32, tag="proj", name="qp_ps"
                        )
                    ps_t = cell[m]
                    for k in range(k0, k1):
                        nc.tensor.matmul(
                            ps_t,
                            wq_sb[:, k, m * 128 : (m + 1) * 128],
                            cell["xq"][:, k, :],
                            start=(k == 0),
                            stop=(k == 7),
                        )
                    if fin:
                        nc.vector.tensor_scalar_add(
                            qT[m][:, tb * TB : (tb + 1) * TB],
                            ps_t,
                            bqk_sb[:, m : m + 1],
                        )

                out = [start]
                for m in range(2):
                    out += [
                        lambda m=m: chunk(m, 0, 3, False),
                        lambda m=m: chunk(m, 3, 6, False),
                        lambda m=m: chunk(m, 6, 8, True),
                    ]
                return out

            def outproj_thunks(tb):
                out = []
                for tt in range(4 * tb, 4 * tb + 4):
                    for nh in range(2):
                        out.append(
                            lambda tt=tt, nh=nh: emit_outproj_nh(tt, nh)
                        )
                return out

            def emit_outproj_nh(tt, nh):
                # evac on DVE (keeps ACT free for the exps) and DMA on the
                # vector HWDGE ring so the sync ring stays dedicated to the
                # big input streams
                ps_t = ps.tile([128, 512], f32, tag="proj", name="op_ps")
                for kk in range(2):
                    nc.tensor.matmul(
                        ps_t,
                        qT[kk][:, tt * 128 : (tt + 1) * 128],
                        wo_sb[:, kk, nh * 512 : (nh + 1) * 512],
                        start=(kk == 0),
                        stop=(kk == 1),
                    )
                o_sb = outs_pool.tile([128, 512], f32, tag="osb", name="o_sb")
                nc.vector.tensor_copy(o_sb, ps_t)
                nc.gpsimd.dma_start(
                    out=OUT[
                        tt * 128 : (tt + 1) * 128, nh * 512 : (nh + 1) * 512
                    ],
                    in_=o_sb,
                )

            def emit_outproj(tb):
                for tt in range(4 * tb, 4 * tb + 4):
                    for nh in range(2):
                        emit_outproj_nh(tt, nh)

            # ---- emission schedule ----
            # preamble: minimal path to the first exp (kb0/m0 + q/m0 +
            # scores kt0..3), then the rest of the projections; steady
            # state: every attention group carries PE filler thunks
            # (qproj of tb+1 or outproj of tb-1) so the PE never runs a
            # matmul-only phase while ACT idles
            xk_ts = [emit_xk(kb_i) for kb_i in range(len(kbs))]
            emit_kproj_m(0, 0, xk_ts[0])
            xq0 = emit_xq(0)
            emit_qproj_m(0, 0, xq0)
            kt_kb0 = min(4, nkt)
            prs0 = [emit_scores(0, 0, kt, "pr0", nkt) for kt in range(kt_kb0)]
            emit_kproj_m(0, 1, xk_ts[0])
            emit_qproj_m(0, 1, xq0)
            for kb_i in range(1, len(kbs)):
                emit_kproj_m(kb_i, 0, xk_ts[kb_i])
                emit_kproj_m(kb_i, 1, xk_ts[kb_i])
            prs0 += [
                emit_scores(0, 0, kt, "pr0", nkt) for kt in range(kt_kb0, nkt)
            ]
            for kt in range(nkt):
                emit_vproj_kt(kt)
            ctx0 = [
                ps.tile([D + 1, TB], f32, tag="ctx", name=f"ctx00_{i}")
                for i in range(2)
            ]
            for kt in range(nkt):
                emit_pv(0, 0, kt, prs0[kt], ctx0)
            emit_normalize(0, 0, ctx0)
            emit_attn(1, 0, thunks=qproj_thunks(1))
            for tb in range(1, NTB):
                emit_attn(0, tb, thunks=outproj_thunks(tb - 1))
                if tb < NTB - 1:
                    emit_attn(1, tb, thunks=qproj_thunks(tb + 1))
                else:
                    emit_attn(1, tb)
            emit_outproj(NTB - 1)

    nc.compile()
    return nc


def kernel(
    query, key, value, Wq, bq, Wk, bk, Wv, bv, Wo, bo, query_mask, key_mask
):
    global LAST_EXEC_NS, LAST_TRACE_DIR
    from concourse.bass_utils import run_bass_kernel_spmd

    query = np.asarray(query, dtype=np.float32)
    key = np.asarray(key, dtype=np.float32)
    value = np.asarray(value, dtype=np.float32)
    Wq = np.asarray(Wq, dtype=np.float32)
    Wk = np.asarray(Wk, dtype=np.float32)
    Wv = np.asarray(Wv, dtype=np.float32)
    Wo = np.asarray(Wo, dtype=np.float32)
    bq = np.asarray(bq, dtype=np.float32)
    bk = np.asarray(bk, dtype=np.float32)
    bv = np.asarray(bv, dtype=np.float32)
    bo = np.asarray(bo, dtype=np.float32)
    qm = np.asarray(query_mask)
    km = np.asarray(key_mask)

    # host-side key compaction (query_mask masks the KEY axis, globally
    # per batch)
    keep = [np.flatnonzero(qm[b] != 0) for b in range(B)]
    nkeep = max((len(k) for k in keep), default=0)
    nkt = max(1, math.ceil(nkeep / 128))
    nkeys = nkt * 128

    with_bv = bool(np.any(bv))
    ck = (nkt, with_bv, BF16)
    if ck not in _CACHE:
        _CACHE[ck] = _build(nkt, with_bv, BF16)
    nc = _CACHE[ck]

    wdt = ml_dtypes.bfloat16 if BF16 else np.float32

    def arr_kmajor(a, ktiles):  # [dim, n] -> [128, ktiles, n]
        return np.ascontiguousarray(
            a.reshape(ktiles, 128, a.shape[1]).transpose(1, 0, 2)
        ).astype(wdt)

    in_maps = []
    for c in range(N_CORES):
        b, hg = c // HG, c % HG
        hs = hg * GH
        idx = keep[b]
        # compacted + padded key/value (transposed)
        xk = np.zeros((KDIM, nkeys), np.float32)
        xk[:, : len(idx)] = key[b].T[:, idx]
        xv = np.zeros((VDIM, nkeys), np.float32)
        xv[:, : len(idx)] = value[b].T[:, idx]
        mbias = np.full((nkeys,), NEG, np.float32)
        mbias[: len(idx)] = 0.0
        bqk = np.empty((128, 4), np.float32)
        bqk[:, 0] = bq[hs : hs + 128]
        bqk[:, 1] = bq[hs + 128 : hs + 256]
        bqk[:, 2] = bk[hs : hs + 128]
        bqk[:, 3] = bk[hs + 128 : hs + 256]
        m = {
            "xq": arr_kmajor(query[b].T, 8),
            "xk": arr_kmajor(xk, 6),
            "xv": arr_kmajor(xv, 6),
            "wq": arr_kmajor(Wq[:, hs : hs + GH], 8),
            "wk": arr_kmajor(Wk[:, hs : hs + GH], 6),
            "wv": arr_kmajor(Wv[:, hs : hs + GH], 6),
            "wo": arr_kmajor(Wo[hs : hs + GH, :], 2),
            "mbias": np.ascontiguousarray(mbias.reshape(nkt, 128).T),
            "bqk": bqk,
        }
        if with_bv:
            bvt = np.empty((128, 2), np.float32)
            bvt[:, 0] = bv[hs : hs + 128]
            bvt[:, 1] = bv[hs + 128 : hs + 256]
            m["bv"] = bvt
        in_maps.append(m)

    kwargs = {}
    if PROFILE:
        import tempfile

        LAST_TRACE_DIR = tempfile.mkdtemp(prefix="bass_trace_")
        kwargs = {"trace": True, "tmpdir": LAST_TRACE_DIR}
    res = run_bass_kernel_spmd(nc, in_maps, list(range(N_CORES)), **kwargs)
    LAST_EXEC_NS = res.exec_time_ns

    out = np.zeros((B, LQ, QDIM), np.float32)
    for c in range(N_CORES):
        out[c // HG] += res.results[c]["outp"]
    out += bo[None, None, :]
    for b in range(B):
        if len(keep[b]) == 0:
            # all keys masked: reference softmax is NaN everywhere
            out[b] = np.nan
    # key_mask masks the QUERY axis in the reference; a zero row makes the
    # whole softmax row -inf -> NaN output for that query position.
    for b in range(B):
        zq = np.flatnonzero(km[b] == 0)
        if len(zq):
            out[b, zq, :] = np.nan
    return out



# revision 21
# speedup vs baseline: 1.0087x; 1.0087x over previous
"""Trainium2 Bass kernel for nn_CrossModalAttention (B=2, LQ=LK=2048,
QDIM=HID=1024, KDIM=VDIM=768, H=16, D=64).

Sharding: 8 cores = 2 batches x 4 head-groups (4 heads each).
Per core: q/k/v projections column-sliced over HID, attention for its 4
heads, row-parallel partial of the out-projection. Host sums the 4
partials per batch (the row-parallel unshard) and adds bo.

Device dataflow (per core), all matmuls in fp32r (TF32-like, ~1.5e-4):
  - host passes query/key/value[b] transposed (and K/V key-compacted:
    query_mask masks the KEY axis globally per batch, so masked keys are
    dropped on host and the remainder padded to a multiple of 128)
  - qT/kT [hid, tokens] and v [keys, hid] computed on device
  - per head pair (row-packed K=64 matmuls via tile_position):
    scoresT [keys, q] -> ACT exp(s/8 + mask_bias) -> PV matmul with a
    ones-augmented V (M=65) giving ctxT and the softmax denominator
  - normalize on DVE (reciprocal + gpsimd partition-broadcast)
  - out-projection from ctxT, partial written to DRAM
"""

import math

import ml_dtypes
import numpy as np

B, LQ, LK = 2, 2048, 2048
QDIM, KDIM, VDIM, HID, H = 1024, 768, 768, 1024, 16
D = HID // H  # 64
HG = 4  # head-groups (cores per batch)
HL = H // HG  # heads per core = 4
GH = HL * D  # per-core hid slice = 256
N_CORES = 8
TB = 512  # token block
NTB = LQ // TB  # 4
NEG = -1.0e30

BF16 = True
PROFILE = False
LAST_EXEC_NS = None
LAST_TRACE_DIR = None

_CACHE = {}


def _build(nkt: int, with_bv: bool, bf16: bool):
    import concourse.bacc as bacc
    import concourse.mybir as mybir
    import concourse.tile as tile

    nkeys = nkt * 128
    # key blocks of <=512 for the k-projection
    kbs = [min(512, nkeys - s) for s in range(0, nkeys, 512)]

    f32 = mybir.dt.float32
    f32r = mybir.dt.bfloat16 if bf16 else mybir.dt.float32r
    Exp = mybir.ActivationFunctionType.Exp
    Ident = mybir.ActivationFunctionType.Identity

    nc = bacc.Bacc(
        "TRN2", target_bir_lowering=False, debug=False, num_devices=N_CORES
    )

    nkb = len(kbs)
    # DRAM tensors (per-core shapes). Inputs are laid out block-major so
    # every DMA moves large contiguous per-partition lines (1KB strided
    # lines are descriptor-rate-bound: ~15us for 1MB)
    XQ = nc.dram_tensor(
        "xq", [128, NTB, 8, TB], f32r, kind="ExternalInput"
    ).ap()
    XK = nc.dram_tensor(
        "xk", [128, nkb, 6, 512], f32r, kind="ExternalInput"
    ).ap()
    XV = nc.dram_tensor(
        "xv", [128, nkb, 6, 512], f32r, kind="ExternalInput"
    ).ap()
    WQ = nc.dram_tensor("wq", [128, 8, GH], f32r, kind="ExternalInput").ap()
    WK = nc.dram_tensor("wk", [128, 6, GH], f32r, kind="ExternalInput").ap()
    WV = nc.dram_tensor("wv", [128, 6, GH], f32r, kind="ExternalInput").ap()
    WO = nc.dram_tensor("wo", [128, 2, QDIM], f32r, kind="ExternalInput").ap()
    MB = nc.dram_tensor("mbias", [128, nkt], f32, kind="ExternalInput").ap()
    BQ = nc.dram_tensor("bqk", [128, 4], f32, kind="ExternalInput").ap()
    BV = None
    if with_bv:
        BV = nc.dram_tensor("bv", [128, 2], f32, kind="ExternalInput").ap()
    # output block-major: [tt, nh, 128, 512] so each store is one fully
    # contiguous 256KB write; the host reassembles
    OUT = nc.dram_tensor(
        "outp", [LQ // 128, 2, 128, 512], f32, kind="ExternalOutput"
    ).ap()

    with tile.TileContext(nc) as tc:
        with (
            tc.tile_pool(name="consts", bufs=1) as consts,
            tc.tile_pool(name="resid", bufs=1) as resid,
            tc.tile_pool(name="xs", bufs=2) as xs,
            tc.tile_pool(name="probs", bufs=4) as probs_pool,
            tc.tile_pool(name="norm", bufs=3) as norm_pool,
            tc.tile_pool(name="outs", bufs=3) as outs_pool,
            tc.tile_pool(name="ps", bufs=2, space="PSUM") as ps,
        ):
            # ---- constants / weights ----
            # weights go on the gpsimd SWDGE ring so the big input streams
            # (sync HWDGE ring) aren't serialized behind them
            wq_sb = consts.tile([128, 8, GH], f32r)
            wk_sb = consts.tile([128, 6, GH], f32r)
            wv_sb = consts.tile([128, 6, GH], f32r)
            wo_sb = consts.tile([128, 2, QDIM], f32r)
            mb_sb = consts.tile([128, nkt], f32)
            bqk_sb = consts.tile([128, 4], f32)
            nc.gpsimd.dma_start(out=wk_sb, in_=WK)
            nc.scalar.dma_start(out=bqk_sb, in_=BQ)
            nc.scalar.dma_start(out=mb_sb, in_=MB)
            nc.gpsimd.dma_start(out=wv_sb, in_=WV)
            nc.gpsimd.dma_start(out=wq_sb, in_=WQ)
            nc.gpsimd.dma_start(out=wo_sb, in_=WO)
            bv_sb = None
            if with_bv:
                bv_sb = consts.tile([128, 2], f32)
                nc.gpsimd.dma_start(out=bv_sb, in_=BV)

            # ---- residents ----
            # qT tiles double as ctxT tiles later (WAR handled by Tile)
            qT = [resid.tile([128, LQ], f32r, tag=f"qT{p}", name=f"qT{p}") for p in range(2)]
            kT = [resid.tile([128, nkeys], f32r, tag=f"kT{p}", name=f"kT{p}") for p in range(2)]
            xq_all = resid.tile([128, NTB, 8, TB], f32r, tag="xq", name="xq")
            v_sb = resid.tile([128, nkt, HL, D + 1], f32r)
            # ones columns for the denominator rows: fill the whole tile,
            # the v-projection copies then overwrite the [., ., ., 0:D] part
            if bf16:
                nc.vector.memset(v_sb, 1.0)
            else:
                nc.vector.memset(v_sb[:, :, :, :].bitcast(f32), 1.0)

            # ---- k projection (per key-block, so attention can begin
            # after kb0) ----
            def emit_xk(kb_i):
                kbw = kbs[kb_i]
                xk_t = xs.tile([128, 6, 512], f32r, tag="xk", name="xk_t", bufs=3)
                nc.sync.dma_start(
                    out=xk_t[:, :, :kbw], in_=XK[:, kb_i, :, :kbw]
                )
                return xk_t

            def emit_kproj_m(kb_i, m, xk_t):
                kbw = kbs[kb_i]
                s0 = kb_i * 512
                ps_t = ps.tile([128, 512], f32, tag="proj", name="kp_ps")
                for k in range(6):
                    nc.tensor.matmul(
                        ps_t[:, :kbw],
                        wk_sb[:, k, m * 128 : (m + 1) * 128],
                        xk_t[:, k, :kbw],
                        start=(k == 0),
                        stop=(k == 5),
                    )
                nc.vector.tensor_scalar_add(
                    kT[m][:, s0 : s0 + kbw],
                    ps_t[:, :kbw],
                    bqk_sb[:, 2 + m : 3 + m],
                )

            # ---- v projection (emitted later for tb0/p0 split; see
            # emit_vproj) ----
            vproj_state = {}

            def emit_xv(kb_i):
                if kb_i not in vproj_state:
                    kbw = kbs[kb_i]
                    xv_t = xs.tile(
                        [128, 6, 512], f32r, tag="xv", name="xv_t",
                        bufs=max(2, len(kbs)),
                    )
                    nc.sync.dma_start(
                        out=xv_t[:, :, :kbw], in_=XV[:, kb_i, :, :kbw]
                    )
                    vproj_state[kb_i] = xv_t
                return vproj_state[kb_i]

            def emit_vproj_kt(kt):
                kb_i = kt // 4
                sub = kt % 4
                xv_t = emit_xv(kb_i)
                ps_t = ps.tile([128, 512], f32, tag="proj", name="vp_ps")
                for k in range(6):
                    nc.tensor.matmul(
                        ps_t[:, :GH],
                        xv_t[:, k, sub * 128 : (sub + 1) * 128],
                        wv_sb[:, k, :],
                        start=(k == 0),
                        stop=(k == 5),
                    )
                nc.vector.tensor_copy(
                    v_sb[:, kt, :, 0:D],
                    ps_t[:, :GH].rearrange("p (h d) -> p h d", h=HL),
                )

            def emit_xq(tb):
                nc.sync.dma_start(out=xq_all[:, tb], in_=XQ[:, tb])

            def emit_qproj_m(tb, m):
                t0 = tb * TB
                ps_t = ps.tile([128, 512], f32, tag="proj", name="qp_ps")
                for k in range(8):
                    nc.tensor.matmul(
                        ps_t,
                        wq_sb[:, k, m * 128 : (m + 1) * 128],
                        xq_all[:, tb, k, :],
                        start=(k == 0),
                        stop=(k == 7),
                    )
                nc.vector.tensor_scalar_add(
                    qT[m][:, t0 : t0 + TB],
                    ps_t,
                    bqk_sb[:, m : m + 1],
                )

            def emit_scores(p, tb, kt, prtag, prbufs):
                t0 = tb * TB
                k0 = kt * 128
                sc = ps.tile([128, 2, TB], f32, tag="sc", name="sc")
                for hh in range(2):
                    nc.tensor.matmul(
                        sc[:, hh, :],
                        kT[p][hh * 64 : hh * 64 + 64, k0 : k0 + 128],
                        qT[p][hh * 64 : hh * 64 + 64, t0 : t0 + TB],
                        start=True,
                        stop=True,
                        tile_position=(hh * 64, 0),
                    )
                pr = probs_pool.tile(
                    [128, 2, TB], f32r, tag=prtag, name="pr", bufs=prbufs
                )
                nc.scalar.activation(
                    pr, sc, Exp, bias=mb_sb[:, kt : kt + 1], scale=0.125
                )
                return pr

            def emit_pv(p, tb, kt, pr, ctx_ps):
                for hh in range(2):
                    nc.tensor.matmul(
                        ctx_ps[hh],
                        v_sb[:, kt, 2 * p + hh, :],
                        pr[:, hh, :],
                        start=(kt == 0),
                        stop=(kt == nkt - 1),
                    )

            def emit_normalize(p, tb, ctx_ps):
                t0 = tb * TB
                # denominator row to SBUF, broadcast across partitions on
                # gpsimd, reciprocal, then one DVE multiply straight out of
                # PSUM into the resident ctx tile (no full-tile evac)
                dcps = []
                for hh in range(2):
                    dcp = norm_pool.tile([1, TB], f32, tag="dcp", name="dcp")
                    nc.vector.tensor_copy(dcp, ctx_ps[hh][D : D + 1, :])
                    dcps.append(dcp)
                for hh in range(2):
                    rbc = norm_pool.tile([D, TB], f32, tag="rbc", name="rbc")
                    nc.gpsimd.partition_broadcast(rbc, dcps[hh])
                    rec = norm_pool.tile([D, TB], f32, tag="rec", name="rec")
                    nc.vector.reciprocal_approx_fast(out=rec, in_=rbc)
                    dst = qT[p][hh * 64 : hh * 64 + 64, t0 : t0 + TB]
                    nc.vector.tensor_mul(dst, ctx_ps[hh][0:D, :], rec)
                    if with_bv:
                        nc.vector.tensor_scalar_add(
                            dst, dst, bv_sb[64 * hh : 64 * hh + 64, p : p + 1]
                        )

            def emit_attn(p, tb, thunks=()):
                # thunk runs between the scores pair and the PV pair of each
                # kt: the interleaved PE work hides the exp(kt) latency the
                # PV would otherwise stall on
                thunks = list(thunks)
                ctx_ps = [
                    ps.tile([D + 1, TB], f32, tag="ctx", name=f"ctx{p}_{tb}_{i}")
                    for i in range(2)
                ]
                for kt in range(nkt):
                    pr = emit_scores(p, tb, kt, "pr", 4)
                    if kt < len(thunks):
                        thunks[kt]()
                    emit_pv(p, tb, kt, pr, ctx_ps)
                for th in thunks[nkt:]:
                    th()
                emit_normalize(p, tb, ctx_ps)

            def qproj_thunks(tb):
                # 6 small thunks: 3+3+2 matmuls per m-half, chained into
                # one psum accumulation (interleaving with other banks is
                # legal), evac on the last
                cell = {}

                def chunk(m, k0, k1, fin, tb=tb):
                    if k0 == 0:
                        cell[m] = ps.tile(
                            [128, 512], f32, tag="proj", name="qp_ps"
                        )
                    ps_t = cell[m]
                    for k in range(k0, k1):
                        nc.tensor.matmul(
                            ps_t,
                            wq_sb[:, k, m * 128 : (m + 1) * 128],
                            xq_all[:, tb, k, :],
                            start=(k == 0),
                            stop=(k == 7),
                        )
                    if fin:
                        nc.vector.tensor_scalar_add(
                            qT[m][:, tb * TB : (tb + 1) * TB],
                            ps_t,
                            bqk_sb[:, m : m + 1],
                        )

                out = []
                for m in range(2):
                    out += [
                        lambda m=m: chunk(m, 0, 3, False),
                        lambda m=m: chunk(m, 3, 6, False),
                        lambda m=m: chunk(m, 6, 8, True),
                    ]
                return out

            def outproj_thunks(tb):
                out = []
                for tt in range(4 * tb, 4 * tb + 4):
                    for nh in range(2):
                        out.append(
                            lambda tt=tt, nh=nh: emit_outproj_nh(tt, nh)
                        )
                return out

            def emit_outproj_nh(tt, nh):
                # evac on DVE (keeps ACT free for the exps) and DMA on the
                # vector HWDGE ring so the sync ring stays dedicated to the
                # big input streams
                ps_t = ps.tile([128, 512], f32, tag="proj", name="op_ps")
                for kk in range(2):
                    nc.tensor.matmul(
                        ps_t,
                        qT[kk][:, tt * 128 : (tt + 1) * 128],
                        wo_sb[:, kk, nh * 512 : (nh + 1) * 512],
                        start=(kk == 0),
                        stop=(kk == 1),
                    )
                o_sb = outs_pool.tile([128, 512], f32, tag="osb", name="o_sb")
                nc.vector.tensor_copy(o_sb, ps_t)
                nc.sync.dma_start(out=OUT[tt, nh], in_=o_sb)

            def emit_outproj(tb):
                for tt in range(4 * tb, 4 * tb + 4):
                    for nh in range(2):
                        emit_outproj_nh(tt, nh)

            # ---- emission schedule ----
            # DMA priority order on the sync ring: xk0, xq0, xk1.., xv
            # blocks, then the remaining xq tiles — everything is issued
            # up front so out-DMAs appended later never delay inputs.
            # Preamble: minimal PE path to the first exp (kb0/m0 + q/m0 +
            # scores kt0..3); steady state: every attention group carries
            # PE filler thunks (qproj of tb+1 or outproj of tb-1) so the
            # PE never runs a matmul-only phase while ACT idles
            xk_ts = [emit_xk(0)]
            emit_xq(0)
            for kb_i in range(1, len(kbs)):
                xk_ts.append(emit_xk(kb_i))
            for kb_i in range(len(kbs)):
                emit_xv(kb_i)
            for tb in range(1, NTB):
                emit_xq(tb)
            emit_kproj_m(0, 0, xk_ts[0])
            emit_qproj_m(0, 0)
            kt_kb0 = min(4, nkt)
            prs0 = [emit_scores(0, 0, kt, "pr0", nkt) for kt in range(kt_kb0)]
            emit_kproj_m(0, 1, xk_ts[0])
            emit_qproj_m(0, 1)
            for kb_i in range(1, len(kbs)):
                emit_kproj_m(kb_i, 0, xk_ts[kb_i])
                emit_kproj_m(kb_i, 1, xk_ts[kb_i])
            prs0 += [
                emit_scores(0, 0, kt, "pr0", nkt) for kt in range(kt_kb0, nkt)
            ]
            for kt in range(nkt):
                emit_vproj_kt(kt)
            ctx0 = [
                ps.tile([D + 1, TB], f32, tag="ctx", name=f"ctx00_{i}")
                for i in range(2)
            ]
            for kt in range(nkt):
                emit_pv(0, 0, kt, prs0[kt], ctx0)
            emit_normalize(0, 0, ctx0)
            emit_attn(1, 0, thunks=qproj_thunks(1))
            for tb in range(1, NTB):
                emit_attn(0, tb, thunks=outproj_thunks(tb - 1))
                if tb < NTB - 1:
                    emit_attn(1, tb, thunks=qproj_thunks(tb + 1))
                else:
                    emit_attn(1, tb)
            emit_outproj(NTB - 1)

    nc.compile()
    return nc


def kernel(
    query, key, value, Wq, bq, Wk, bk, Wv, bv, Wo, bo, query_mask, key_mask
):
    global LAST_EXEC_NS, LAST_TRACE_DIR
    from concourse.bass_utils import run_bass_kernel_spmd

    query = np.asarray(query, dtype=np.float32)
    key = np.asarray(key, dtype=np.float32)
    value = np.asarray(value, dtype=np.float32)
    Wq = np.asarray(Wq, dtype=np.float32)
    Wk = np.asarray(Wk, dtype=np.float32)
    Wv = np.asarray(Wv, dtype=np.float32)
    Wo = np.asarray(Wo, dtype=np.float32)
    bq = np.asarray(bq, dtype=np.float32)
    bk = np.asarray(bk, dtype=np.float32)
    bv = np.asarray(bv, dtype=np.float32)
    bo = np.asarray(bo, dtype=np.float32)
    qm = np.asarray(query_mask)
    km = np.asarray(key_mask)

    # host-side key compaction (query_mask masks the KEY axis, globally
    # per batch)
    keep = [np.flatnonzero(qm[b] != 0) for b in range(B)]
    nkeep = max((len(k) for k in keep), default=0)
    nkt = max(1, math.ceil(nkeep / 128))
    nkeys = nkt * 128

    with_bv = bool(np.any(bv))
    ck = (nkt, with_bv, BF16)
    if ck not in _CACHE:
        _CACHE[ck] = _build(nkt, with_bv, BF16)
    nc = _CACHE[ck]

    wdt = ml_dtypes.bfloat16 if BF16 else np.float32
    nkb = math.ceil(nkeys / 512)
    nkeys_b = nkb * 512

    def arr_kmajor(a, ktiles):  # [dim, n] -> [128, ktiles, n]
        return np.ascontiguousarray(
            a.reshape(ktiles, 128, a.shape[1]).transpose(1, 0, 2)
        ).astype(wdt)

    def arr_blocked(a, ktiles, blocks):  # [dim, n] -> [128, blocks, kt, 512]
        return np.ascontiguousarray(
            a.reshape(ktiles, 128, blocks, 512).transpose(1, 2, 0, 3)
        ).astype(wdt)

    in_maps = []
    for c in range(N_CORES):
        b, hg = c // HG, c % HG
        hs = hg * GH
        idx = keep[b]
        # compacted + padded key/value (transposed)
        xk = np.zeros((KDIM, nkeys_b), np.float32)
        xk[:, : len(idx)] = key[b].T[:, idx]
        xv = np.zeros((VDIM, nkeys_b), np.float32)
        xv[:, : len(idx)] = value[b].T[:, idx]
        mbias = np.full((nkeys,), NEG, np.float32)
        mbias[: len(idx)] = 0.0
        bqk = np.empty((128, 4), np.float32)
        bqk[:, 0] = bq[hs : hs + 128]
        bqk[:, 1] = bq[hs + 128 : hs + 256]
        bqk[:, 2] = bk[hs : hs + 128]
        bqk[:, 3] = bk[hs + 128 : hs + 256]
        m = {
            "xq": arr_blocked(query[b].T, 8, NTB),
            "xk": arr_blocked(xk, 6, nkb),
            "xv": arr_blocked(xv, 6, nkb),
            "wq": arr_kmajor(Wq[:, hs : hs + GH], 8),
            "wk": arr_kmajor(Wk[:, hs : hs + GH], 6),
            "wv": arr_kmajor(Wv[:, hs : hs + GH], 6),
            "wo": arr_kmajor(Wo[hs : hs + GH, :], 2),
            "mbias": np.ascontiguousarray(mbias.reshape(nkt, 128).T),
            "bqk": bqk,
        }
        if with_bv:
            bvt = np.empty((128, 2), np.float32)
            bvt[:, 0] = bv[hs : hs + 128]
            bvt[:, 1] = bv[hs + 128 : hs + 256]
            m["bv"] = bvt
        in_maps.append(m)

    kwargs = {}
    if PROFILE:
        import tempfile

        LAST_TRACE_DIR = tempfile.mkdtemp(prefix="bass_trace_")
        kwargs = {"trace": True, "tmpdir": LAST_TRACE_DIR}
    res = run_bass_kernel_spmd(nc, in_maps, list(range(N_CORES)), **kwargs)
    LAST_EXEC_NS = res.exec_time_ns

    out = np.zeros((B, LQ, QDIM), np.float32)
    for c in range(N_CORES):
        blk = res.results[c]["outp"]  # [LQ//128, 2, 128, 512] block-major
        out[c // HG] += blk.transpose(0, 2, 1, 3).reshape(LQ, QDIM)
    out += bo[None, None, :]
    for b in range(B):
        if len(keep[b]) == 0:
            # all keys masked: reference softmax is NaN everywhere
            out[b] = np.nan
    # key_mask masks the QUERY axis in the reference; a zero row makes the
    # whole softmax row -inf -> NaN output for that query position.
    for b in range(B):
        zq = np.flatnonzero(km[b] == 0)
        if len(zq):
            out[b, zq, :] = np.nan
    return out



# revision 25
# speedup vs baseline: 1.0520x; 1.0428x over previous
"""Trainium2 Bass kernel for nn_CrossModalAttention (B=2, LQ=LK=2048,
QDIM=HID=1024, KDIM=VDIM=768, H=16, D=64).

Sharding: 8 cores = 2 batches x 4 head-groups (4 heads each).
Per core: q/k/v projections column-sliced over HID, attention for its 4
heads, row-parallel partial of the out-projection. Host sums the 4
partials per batch (the row-parallel unshard) and adds bo.

Device dataflow (per core), all matmuls in fp32r (TF32-like, ~1.5e-4):
  - host passes query/key/value[b] transposed (and K/V key-compacted:
    query_mask masks the KEY axis globally per batch, so masked keys are
    dropped on host and the remainder padded to a multiple of 128)
  - qT/kT [hid, tokens] and v [keys, hid] computed on device
  - per head pair (row-packed K=64 matmuls via tile_position):
    scoresT [keys, q] -> ACT exp(s/8 + mask_bias) -> PV matmul with a
    ones-augmented V (M=65) giving ctxT and the softmax denominator
  - normalize on DVE (reciprocal + gpsimd partition-broadcast)
  - out-projection from ctxT, partial written to DRAM
"""

import math

import ml_dtypes
import numpy as np

B, LQ, LK = 2, 2048, 2048
QDIM, KDIM, VDIM, HID, H = 1024, 768, 768, 1024, 16
D = HID // H  # 64
HG = 4  # head-groups (cores per batch)
HL = H // HG  # heads per core = 4
GH = HL * D  # per-core hid slice = 256
N_CORES = 8
TB = 512  # token block
NTB = LQ // TB  # 4
NEG = -1.0e30

BF16 = True
PROFILE = False
LAST_EXEC_NS = None
LAST_TRACE_DIR = None

_CACHE = {}


def _build(nkt: int, with_bv: bool, bf16: bool):
    import concourse.bacc as bacc
    import concourse.mybir as mybir
    import concourse.tile as tile

    nkeys = nkt * 128
    # key blocks of <=512 for the k-projection
    kbs = [min(512, nkeys - s) for s in range(0, nkeys, 512)]

    f32 = mybir.dt.float32
    f32r = mybir.dt.bfloat16 if bf16 else mybir.dt.float32r
    Exp = mybir.ActivationFunctionType.Exp
    Ident = mybir.ActivationFunctionType.Identity

    nc = bacc.Bacc(
        "TRN2", target_bir_lowering=False, debug=False, num_devices=N_CORES
    )

    nkb = len(kbs)
    # DRAM tensors (per-core shapes). Inputs are laid out block-major so
    # every DMA moves large contiguous per-partition lines (1KB strided
    # lines are descriptor-rate-bound: ~15us for 1MB)
    XQ = nc.dram_tensor(
        "xq", [128, NTB, 8, TB], f32r, kind="ExternalInput"
    ).ap()
    XK = nc.dram_tensor(
        "xk", [128, nkb, 6, 512], f32r, kind="ExternalInput"
    ).ap()
    XV = nc.dram_tensor(
        "xv", [128, nkb, 6, 512], f32r, kind="ExternalInput"
    ).ap()
    WQ = nc.dram_tensor("wq", [128, 8, GH], f32r, kind="ExternalInput").ap()
    WK = nc.dram_tensor("wk", [128, 6, GH], f32r, kind="ExternalInput").ap()
    WV = nc.dram_tensor("wv", [128, 6, GH], f32r, kind="ExternalInput").ap()
    WO = nc.dram_tensor("wo", [128, 2, QDIM], f32r, kind="ExternalInput").ap()
    MB = nc.dram_tensor("mbias", [128, nkt], f32, kind="ExternalInput").ap()
    BQ = nc.dram_tensor("bqk", [128, 4], f32, kind="ExternalInput").ap()
    BV = None
    if with_bv:
        BV = nc.dram_tensor("bv", [128, 2], f32, kind="ExternalInput").ap()
    # output block-major: [tt, nh, 128, 512] so each store is one fully
    # contiguous 256KB write; the host reassembles
    OUT = nc.dram_tensor(
        "outp", [LQ // 128, 2, 128, 512], f32, kind="ExternalOutput"
    ).ap()

    with tile.TileContext(nc) as tc:
        with (
            tc.tile_pool(name="consts", bufs=1) as consts,
            tc.tile_pool(name="resid", bufs=1) as resid,
            tc.tile_pool(name="xs", bufs=2) as xs,
            tc.tile_pool(name="probs", bufs=4) as probs_pool,
            tc.tile_pool(name="norm", bufs=3) as norm_pool,
            tc.tile_pool(name="outs", bufs=3) as outs_pool,
            tc.tile_pool(name="ps", bufs=2, space="PSUM") as ps,
        ):
            # ---- constants / weights ----
            # wk/wq gate the very first matmuls: they go FIRST on the sync
            # HWDGE ring (the gpsimd SWDGE ring takes ~7us to complete a
            # transfer, which used to stall the PE until ~15us). wv/wo are
            # needed later and stay on the SWDGE ring.
            wq_sb = consts.tile([128, 8, GH], f32r)
            wk_sb = consts.tile([128, 6, GH], f32r)
            wv_sb = consts.tile([128, 6, GH], f32r)
            wo_sb = consts.tile([128, 2, QDIM], f32r)
            mb_sb = consts.tile([128, nkt], f32)
            bqk_sb = consts.tile([128, 4], f32)
            nc.sync.dma_start(out=wk_sb, in_=WK)
            nc.scalar.dma_start(out=bqk_sb, in_=BQ)
            nc.scalar.dma_start(out=mb_sb, in_=MB)
            nc.gpsimd.dma_start(out=wv_sb, in_=WV)
            nc.gpsimd.dma_start(out=wo_sb, in_=WO)
            bv_sb = None
            if with_bv:
                bv_sb = consts.tile([128, 2], f32)
                nc.gpsimd.dma_start(out=bv_sb, in_=BV)

            # ---- residents ----
            # qT tiles double as ctxT tiles later (WAR handled by Tile)
            qT = [resid.tile([128, LQ], f32r, tag=f"qT{p}", name=f"qT{p}") for p in range(2)]
            kT = [resid.tile([128, nkeys], f32r, tag=f"kT{p}", name=f"kT{p}") for p in range(2)]
            xq_all = resid.tile([128, NTB, 8, TB], f32r, tag="xq", name="xq")
            v_sb = resid.tile([128, nkt, HL, D + 1], f32r)
            # ones columns for the denominator rows: fill the whole tile,
            # the v-projection copies then overwrite the [., ., ., 0:D] part
            if bf16:
                nc.vector.memset(v_sb, 1.0)
            else:
                nc.vector.memset(v_sb[:, :, :, :].bitcast(f32), 1.0)

            # ---- k projection (per key-block, so attention can begin
            # after kb0) ----
            def emit_xk(kb_i):
                kbw = kbs[kb_i]
                xk_t = xs.tile([128, 6, 512], f32r, tag="xk", name="xk_t", bufs=3)
                nc.sync.dma_start(
                    out=xk_t[:, :, :kbw], in_=XK[:, kb_i, :, :kbw]
                )
                return xk_t

            def emit_kproj_m(kb_i, m, xk_t):
                kbw = kbs[kb_i]
                s0 = kb_i * 512
                ps_t = ps.tile([128, 512], f32, tag="proj", name="kp_ps")
                for k in range(6):
                    nc.tensor.matmul(
                        ps_t[:, :kbw],
                        wk_sb[:, k, m * 128 : (m + 1) * 128],
                        xk_t[:, k, :kbw],
                        start=(k == 0),
                        stop=(k == 5),
                    )
                nc.vector.tensor_scalar_add(
                    kT[m][:, s0 : s0 + kbw],
                    ps_t[:, :kbw],
                    bqk_sb[:, 2 + m : 3 + m],
                )

            # ---- v projection (emitted later for tb0/p0 split; see
            # emit_vproj) ----
            vproj_state = {}

            def emit_xv(kb_i):
                if kb_i not in vproj_state:
                    kbw = kbs[kb_i]
                    xv_t = xs.tile(
                        [128, 6, 512], f32r, tag="xv", name="xv_t",
                        bufs=max(2, len(kbs)),
                    )
                    nc.sync.dma_start(
                        out=xv_t[:, :, :kbw], in_=XV[:, kb_i, :, :kbw]
                    )
                    vproj_state[kb_i] = xv_t
                return vproj_state[kb_i]

            def emit_vproj_kt(kt):
                kb_i = kt // 4
                sub = kt % 4
                xv_t = emit_xv(kb_i)
                ps_t = ps.tile([128, 512], f32, tag="proj", name="vp_ps")
                for k in range(6):
                    nc.tensor.matmul(
                        ps_t[:, :GH],
                        xv_t[:, k, sub * 128 : (sub + 1) * 128],
                        wv_sb[:, k, :],
                        start=(k == 0),
                        stop=(k == 5),
                    )
                nc.vector.tensor_copy(
                    v_sb[:, kt, :, 0:D],
                    ps_t[:, :GH].rearrange("p (h d) -> p h d", h=HL),
                )

            def emit_xq(tb):
                nc.sync.dma_start(out=xq_all[:, tb], in_=XQ[:, tb])

            def emit_qproj_m(tb, m):
                t0 = tb * TB
                ps_t = ps.tile([128, 512], f32, tag="proj", name="qp_ps")
                for k in range(8):
                    nc.tensor.matmul(
                        ps_t,
                        wq_sb[:, k, m * 128 : (m + 1) * 128],
                        xq_all[:, tb, k, :],
                        start=(k == 0),
                        stop=(k == 7),
                    )
                nc.vector.tensor_scalar_add(
                    qT[m][:, t0 : t0 + TB],
                    ps_t,
                    bqk_sb[:, m : m + 1],
                )

            def emit_scores(p, tb, kt, prtag, prbufs):
                t0 = tb * TB
                k0 = kt * 128
                sc = ps.tile([128, 2, TB], f32, tag="sc", name="sc")
                for hh in range(2):
                    nc.tensor.matmul(
                        sc[:, hh, :],
                        kT[p][hh * 64 : hh * 64 + 64, k0 : k0 + 128],
                        qT[p][hh * 64 : hh * 64 + 64, t0 : t0 + TB],
                        start=True,
                        stop=True,
                        tile_position=(hh * 64, 0),
                    )
                pr = probs_pool.tile(
                    [128, 2, TB], f32r, tag=prtag, name="pr", bufs=prbufs
                )
                nc.scalar.activation(
                    pr, sc, Exp, bias=mb_sb[:, kt : kt + 1], scale=0.125
                )
                return pr

            def emit_pv(p, tb, kt, pr, ctx_ps):
                for hh in range(2):
                    nc.tensor.matmul(
                        ctx_ps[hh],
                        v_sb[:, kt, 2 * p + hh, :],
                        pr[:, hh, :],
                        start=(kt == 0),
                        stop=(kt == nkt - 1),
                    )

            def emit_normalize(p, tb, ctx_ps):
                t0 = tb * TB
                # evacuate both PSUM ctx tiles to SBUF first (quick DVE
                # copies) so the next block's PV matmuls get their PSUM
                # slots immediately; the normalize chain then runs from
                # SBUF off the PE critical path
                evac = []
                for hh in range(2):
                    ctmp = norm_pool.tile([D, TB], f32, tag="ctmp", name="ctmp")
                    nc.vector.tensor_copy(ctmp, ctx_ps[hh][0:D, :])
                    dcp = norm_pool.tile([1, TB], f32, tag="dcp", name="dcp")
                    nc.vector.tensor_copy(dcp, ctx_ps[hh][D : D + 1, :])
                    evac.append((ctmp, dcp))
                for hh in range(2):
                    ctmp, dcp = evac[hh]
                    rbc = norm_pool.tile([D, TB], f32, tag="rbc", name="rbc")
                    nc.gpsimd.partition_broadcast(rbc, dcp)
                    rec = norm_pool.tile([D, TB], f32, tag="rec", name="rec")
                    nc.vector.reciprocal_approx_fast(out=rec, in_=rbc)
                    dst = qT[p][hh * 64 : hh * 64 + 64, t0 : t0 + TB]
                    nc.vector.tensor_mul(dst, ctmp, rec)
                    if with_bv:
                        nc.vector.tensor_scalar_add(
                            dst, dst, bv_sb[64 * hh : 64 * hh + 64, p : p + 1]
                        )

            def emit_attn(p, tb, thunks=()):
                # thunk runs between the scores pair and the PV pair of each
                # kt: the interleaved PE work hides the exp(kt) latency the
                # PV would otherwise stall on
                thunks = list(thunks)
                ctx_ps = [
                    ps.tile([D + 1, TB], f32, tag="ctx", name=f"ctx{p}_{tb}_{i}")
                    for i in range(2)
                ]
                for kt in range(nkt):
                    pr = emit_scores(p, tb, kt, "pr", 4)
                    if kt < len(thunks):
                        thunks[kt]()
                    emit_pv(p, tb, kt, pr, ctx_ps)
                for th in thunks[nkt:]:
                    th()
                emit_normalize(p, tb, ctx_ps)

            def qproj_thunks(tb):
                # 6 small thunks: 3+3+2 matmuls per m-half, chained into
                # one psum accumulation (interleaving with other banks is
                # legal), evac on the last
                cell = {}

                def chunk(m, k0, k1, fin, tb=tb):
                    if k0 == 0:
                        cell[m] = ps.tile(
                            [128, 512], f32, tag="proj", name="qp_ps"
                        )
                    ps_t = cell[m]
                    for k in range(k0, k1):
                        nc.tensor.matmul(
                            ps_t,
                            wq_sb[:, k, m * 128 : (m + 1) * 128],
                            xq_all[:, tb, k, :],
                            start=(k == 0),
                            stop=(k == 7),
                        )
                    if fin:
                        nc.vector.tensor_scalar_add(
                            qT[m][:, tb * TB : (tb + 1) * TB],
                            ps_t,
                            bqk_sb[:, m : m + 1],
                        )

                out = []
                for m in range(2):
                    out += [
                        lambda m=m: chunk(m, 0, 3, False),
                        lambda m=m: chunk(m, 3, 6, False),
                        lambda m=m: chunk(m, 6, 8, True),
                    ]
                return out

            def outproj_thunks(tb):
                # two leading no-ops: the first pieces otherwise stall the
                # in-order PE queue on the normalize (DVE+gpsimd chain) of
                # the block whose ctx they consume, starving ACT
                out = [lambda: None, lambda: None]
                for tt in range(4 * tb, 4 * tb + 4):
                    for nh in range(2):
                        out.append(
                            lambda tt=tt, nh=nh: emit_outproj_nh(tt, nh)
                        )
                return out

            def emit_outproj_nh(tt, nh):
                # evac on DVE (keeps ACT free for the exps) and DMA on the
                # vector HWDGE ring so the sync ring stays dedicated to the
                # big input streams
                ps_t = ps.tile([128, 512], f32, tag="proj", name="op_ps")
                for kk in range(2):
                    nc.tensor.matmul(
                        ps_t,
                        qT[kk][:, tt * 128 : (tt + 1) * 128],
                        wo_sb[:, kk, nh * 512 : (nh + 1) * 512],
                        start=(kk == 0),
                        stop=(kk == 1),
                    )
                o_sb = outs_pool.tile([128, 512], f32, tag="osb", name="o_sb")
                nc.vector.tensor_copy(o_sb, ps_t)
                nc.sync.dma_start(out=OUT[tt, nh], in_=o_sb)

            def emit_outproj(tb):
                for tt in range(4 * tb, 4 * tb + 4):
                    for nh in range(2):
                        emit_outproj_nh(tt, nh)

            # ---- emission schedule ----
            # DMA priority order on the sync ring: xk0, xq0, xk1.., xv
            # blocks, then the remaining xq tiles — everything is issued
            # up front so out-DMAs appended later never delay inputs.
            # Preamble: minimal PE path to the first exp (kb0/m0 + q/m0 +
            # scores kt0..3); steady state: every attention group carries
            # PE filler thunks (qproj of tb+1 or outproj of tb-1) so the
            # PE never runs a matmul-only phase while ACT idles
            xk_ts = [emit_xk(0)]
            nc.sync.dma_start(out=wq_sb, in_=WQ)
            emit_xq(0)
            for kb_i in range(1, len(kbs)):
                xk_ts.append(emit_xk(kb_i))
            for kb_i in range(len(kbs)):
                emit_xv(kb_i)
            for tb in range(1, NTB):
                emit_xq(tb)
            emit_kproj_m(0, 0, xk_ts[0])
            emit_qproj_m(0, 0)
            kt_kb0 = min(4, nkt)
            prs0 = [emit_scores(0, 0, kt, "pr0", nkt) for kt in range(kt_kb0)]
            emit_kproj_m(0, 1, xk_ts[0])
            emit_qproj_m(0, 1)
            for kb_i in range(1, len(kbs)):
                emit_kproj_m(kb_i, 0, xk_ts[kb_i])
                emit_kproj_m(kb_i, 1, xk_ts[kb_i])
            prs0 += [
                emit_scores(0, 0, kt, "pr0", nkt) for kt in range(kt_kb0, nkt)
            ]
            for kt in range(nkt):
                emit_vproj_kt(kt)
            ctx0 = [
                ps.tile([D + 1, TB], f32, tag="ctx", name=f"ctx00_{i}")
                for i in range(2)
            ]
            for kt in range(nkt):
                emit_pv(0, 0, kt, prs0[kt], ctx0)
            emit_normalize(0, 0, ctx0)
            emit_attn(1, 0, thunks=qproj_thunks(1))
            for tb in range(1, NTB):
                emit_attn(0, tb, thunks=outproj_thunks(tb - 1))
                if tb < NTB - 1:
                    emit_attn(1, tb, thunks=qproj_thunks(tb + 1))
                else:
                    emit_attn(1, tb)
            emit_outproj(NTB - 1)

    nc.compile()
    return nc


def kernel(
    query, key, value, Wq, bq, Wk, bk, Wv, bv, Wo, bo, query_mask, key_mask
):
    global LAST_EXEC_NS, LAST_TRACE_DIR
    from concourse.bass_utils import run_bass_kernel_spmd

    query = np.asarray(query, dtype=np.float32)
    key = np.asarray(key, dtype=np.float32)
    value = np.asarray(value, dtype=np.float32)
    Wq = np.asarray(Wq, dtype=np.float32)
    Wk = np.asarray(Wk, dtype=np.float32)
    Wv = np.asarray(Wv, dtype=np.float32)
    Wo = np.asarray(Wo, dtype=np.float32)
    bq = np.asarray(bq, dtype=np.float32)
    bk = np.asarray(bk, dtype=np.float32)
    bv = np.asarray(bv, dtype=np.float32)
    bo = np.asarray(bo, dtype=np.float32)
    qm = np.asarray(query_mask)
    km = np.asarray(key_mask)

    # host-side key compaction (query_mask masks the KEY axis, globally
    # per batch)
    keep = [np.flatnonzero(qm[b] != 0) for b in range(B)]
    nkeep = max((len(k) for k in keep), default=0)
    nkt = max(1, math.ceil(nkeep / 128))
    nkeys = nkt * 128

    with_bv = bool(np.any(bv))
    ck = (nkt, with_bv, BF16)
    if ck not in _CACHE:
        _CACHE[ck] = _build(nkt, with_bv, BF16)
    nc = _CACHE[ck]

    wdt = ml_dtypes.bfloat16 if BF16 else np.float32
    nkb = math.ceil(nkeys / 512)
    nkeys_b = nkb * 512

    def arr_kmajor(a, ktiles):  # [dim, n] -> [128, ktiles, n]
        return np.ascontiguousarray(
            a.reshape(ktiles, 128, a.shape[1]).transpose(1, 0, 2)
        ).astype(wdt)

    def arr_blocked(a, ktiles, blocks):  # [dim, n] -> [128, blocks, kt, 512]
        return np.ascontiguousarray(
            a.reshape(ktiles, 128, blocks, 512).transpose(1, 2, 0, 3)
        ).astype(wdt)

    in_maps = []
    for c in range(N_CORES):
        b, hg = c // HG, c % HG
        hs = hg * GH
        idx = keep[b]
        # compacted + padded key/value (transposed)
        xk = np.zeros((KDIM, nkeys_b), np.float32)
        xk[:, : len(idx)] = key[b].T[:, idx]
        xv = np.zeros((VDIM, nkeys_b), np.float32)
        xv[:, : len(idx)] = value[b].T[:, idx]
        mbias = np.full((nkeys,), NEG, np.float32)
        mbias[: len(idx)] = 0.0
        bqk = np.empty((128, 4), np.float32)
        bqk[:, 0] = bq[hs : hs + 128]
        bqk[:, 1] = bq[hs + 128 : hs + 256]
        bqk[:, 2] = bk[hs : hs + 128]
        bqk[:, 3] = bk[hs + 128 : hs + 256]
        m = {
            "xq": arr_blocked(query[b].T, 8, NTB),
            "xk": arr_blocked(xk, 6, nkb),
            "xv": arr_blocked(xv, 6, nkb),
            "wq": arr_kmajor(Wq[:, hs : hs + GH], 8),
            "wk": arr_kmajor(Wk[:, hs : hs + GH], 6),
            "wv": arr_kmajor(Wv[:, hs : hs + GH], 6),
            "wo": arr_kmajor(Wo[hs : hs + GH, :], 2),
            "mbias": np.ascontiguousarray(mbias.reshape(nkt, 128).T),
            "bqk": bqk,
        }
        if with_bv:
            bvt = np.empty((128, 2), np.float32)
            bvt[:, 0] = bv[hs : hs + 128]
            bvt[:, 1] = bv[hs + 128 : hs + 256]
            m["bv"] = bvt
        in_maps.append(m)

    kwargs = {}
    if PROFILE:
        import tempfile

        LAST_TRACE_DIR = tempfile.mkdtemp(prefix="bass_trace_")
        kwargs = {"trace": True, "tmpdir": LAST_TRACE_DIR}
    res = run_bass_kernel_spmd(nc, in_maps, list(range(N_CORES)), **kwargs)
    LAST_EXEC_NS = res.exec_time_ns

    out = np.zeros((B, LQ, QDIM), np.float32)
    for c in range(N_CORES):
        blk = res.results[c]["outp"]  # [LQ//128, 2, 128, 512] block-major
        out[c // HG] += blk.transpose(0, 2, 1, 3).reshape(LQ, QDIM)
    out += bo[None, None, :]
    for b in range(B):
        if len(keep[b]) == 0:
            # all keys masked: reference softmax is NaN everywhere
            out[b] = np.nan
    # key_mask masks the QUERY axis in the reference; a zero row makes the
    # whole softmax row -inf -> NaN output for that query position.
    for b in range(B):
        zq = np.flatnonzero(km[b] == 0)
        if len(zq):
            out[b, zq, :] = np.nan
    return out



# revision 30
# speedup vs baseline: 1.1144x; 1.0594x over previous
"""Trainium2 Bass kernel for nn_CrossModalAttention (B=2, LQ=LK=2048,
QDIM=HID=1024, KDIM=VDIM=768, H=16, D=64).

Sharding: 8 cores = 2 batches x 4 head-groups (4 heads each).
Per core: q/k/v projections column-sliced over HID, attention for its 4
heads, row-parallel partial of the out-projection. Host sums the 4
partials per batch (the row-parallel unshard) and adds bo.

Device dataflow (per core), all matmuls in fp32r (TF32-like, ~1.5e-4):
  - host passes query/key/value[b] transposed (and K/V key-compacted:
    query_mask masks the KEY axis globally per batch, so masked keys are
    dropped on host and the remainder padded to a multiple of 128)
  - qT/kT [hid, tokens] and v [keys, hid] computed on device
  - per head pair (row-packed K=64 matmuls via tile_position):
    scoresT [keys, q] -> ACT exp(s/8 + mask_bias) -> PV matmul with a
    ones-augmented V (M=65) giving ctxT and the softmax denominator
  - normalize on DVE (reciprocal + gpsimd partition-broadcast)
  - out-projection from ctxT, partial written to DRAM
"""

import math

import ml_dtypes
import numpy as np

B, LQ, LK = 2, 2048, 2048
QDIM, KDIM, VDIM, HID, H = 1024, 768, 768, 1024, 16
D = HID // H  # 64
HG = 4  # head-groups (cores per batch)
HL = H // HG  # heads per core = 4
GH = HL * D  # per-core hid slice = 256
N_CORES = 8
TB = 512  # token block
NTB = LQ // TB  # 4
NEG = -1.0e30

BF16 = True
PROFILE = False
LAST_EXEC_NS = None
LAST_TRACE_DIR = None

_CACHE = {}


def _build(nkt: int, with_bv: bool, bf16: bool):
    import concourse.bacc as bacc
    import concourse.mybir as mybir
    import concourse.tile as tile

    nkeys = nkt * 128
    # key blocks of <=512 for the k-projection
    kbs = [min(512, nkeys - s) for s in range(0, nkeys, 512)]

    f32 = mybir.dt.float32
    f32r = mybir.dt.bfloat16 if bf16 else mybir.dt.float32r
    Exp = mybir.ActivationFunctionType.Exp
    Ident = mybir.ActivationFunctionType.Identity

    nc = bacc.Bacc(
        "TRN2", target_bir_lowering=False, debug=False, num_devices=N_CORES
    )

    nkb = len(kbs)
    # DRAM tensors (per-core shapes). Inputs are laid out block-major so
    # every DMA moves large contiguous per-partition lines (1KB strided
    # lines are descriptor-rate-bound: ~15us for 1MB)
    XQ = nc.dram_tensor(
        "xq", [128, NTB, 8, TB], f32r, kind="ExternalInput"
    ).ap()
    XK = nc.dram_tensor(
        "xk", [128, nkb, 6, 512], f32r, kind="ExternalInput"
    ).ap()
    XV = nc.dram_tensor(
        "xv", [128, nkb, 6, 512], f32r, kind="ExternalInput"
    ).ap()
    WQ = nc.dram_tensor("wq", [128, 8, GH], f32r, kind="ExternalInput").ap()
    WK = nc.dram_tensor("wk", [128, 6, GH], f32r, kind="ExternalInput").ap()
    WV = nc.dram_tensor("wv", [128, 6, GH], f32r, kind="ExternalInput").ap()
    WO = nc.dram_tensor("wo", [128, 2, QDIM], f32r, kind="ExternalInput").ap()
    MB = nc.dram_tensor("mbias", [128, nkt], f32, kind="ExternalInput").ap()
    BQ = nc.dram_tensor("bqk", [128, 4], f32, kind="ExternalInput").ap()
    BV = None
    if with_bv:
        BV = nc.dram_tensor("bv", [128, 2], f32, kind="ExternalInput").ap()
    # output block-major: [tt, nh, 128, 512] so each store is one fully
    # contiguous write; bf16 halves the drain (host sums partials in f32)
    OUT = nc.dram_tensor(
        "outp", [LQ // 128, 2, 128, 512], f32r, kind="ExternalOutput"
    ).ap()

    with tile.TileContext(nc) as tc:
        with (
            tc.tile_pool(name="consts", bufs=1) as consts,
            tc.tile_pool(name="resid", bufs=1) as resid,
            tc.tile_pool(name="xs", bufs=2) as xs,
            tc.tile_pool(name="probs", bufs=4) as probs_pool,
            tc.tile_pool(name="norm", bufs=3) as norm_pool,
            tc.tile_pool(name="outs", bufs=3) as outs_pool,
            tc.tile_pool(name="ps", bufs=2, space="PSUM") as ps,
        ):
            # ---- constants / weights ----
            # wk/wq gate the very first matmuls: they go FIRST on the sync
            # HWDGE ring (the gpsimd SWDGE ring takes ~7us to complete a
            # transfer, which used to stall the PE until ~15us). wv/wo are
            # needed later and stay on the SWDGE ring.
            wq_sb = consts.tile([128, 8, GH], f32r)
            wk_sb = consts.tile([128, 6, GH], f32r)
            wv_sb = consts.tile([128, 6, GH], f32r)
            wo_sb = consts.tile([128, 2, QDIM], f32r)
            mb_sb = consts.tile([128, nkt], f32)
            bqk_sb = consts.tile([128, 4], f32)
            nc.sync.dma_start(out=wk_sb, in_=WK)
            nc.scalar.dma_start(out=bqk_sb, in_=BQ)
            nc.scalar.dma_start(out=mb_sb, in_=MB)
            nc.gpsimd.dma_start(out=wv_sb, in_=WV)
            nc.gpsimd.dma_start(out=wo_sb, in_=WO)
            bv_sb = None
            if with_bv:
                bv_sb = consts.tile([128, 2], f32)
                nc.gpsimd.dma_start(out=bv_sb, in_=BV)

            # ---- residents ----
            # qT tiles double as ctxT tiles later (WAR handled by Tile)
            qT = [resid.tile([128, LQ], f32r, tag=f"qT{p}", name=f"qT{p}") for p in range(2)]
            kT = [resid.tile([128, nkeys], f32r, tag=f"kT{p}", name=f"kT{p}") for p in range(2)]
            xq_all = resid.tile([128, NTB, 8, TB], f32r, tag="xq", name="xq")
            v_sb = resid.tile([128, nkt, HL, D + 1], f32r)
            # ones columns for the denominator rows: fill the whole tile,
            # the v-projection copies then overwrite the [., ., ., 0:D] part
            if bf16:
                nc.vector.memset(v_sb, 1.0)
            else:
                nc.vector.memset(v_sb[:, :, :, :].bitcast(f32), 1.0)

            # ---- k projection (per key-block, so attention can begin
            # after kb0) ----
            def emit_xk(kb_i):
                kbw = kbs[kb_i]
                xk_t = xs.tile([128, 6, 512], f32r, tag="xk", name="xk_t", bufs=3)
                nc.sync.dma_start(
                    out=xk_t[:, :, :kbw], in_=XK[:, kb_i, :, :kbw]
                )
                return xk_t

            def emit_kproj_m(kb_i, m, xk_t):
                kbw = kbs[kb_i]
                s0 = kb_i * 512
                ps_t = ps.tile([128, 512], f32, tag="proj", name="kp_ps")
                for k in range(6):
                    nc.tensor.matmul(
                        ps_t[:, :kbw],
                        wk_sb[:, k, m * 128 : (m + 1) * 128],
                        xk_t[:, k, :kbw],
                        start=(k == 0),
                        stop=(k == 5),
                    )
                nc.vector.tensor_scalar_add(
                    kT[m][:, s0 : s0 + kbw],
                    ps_t[:, :kbw],
                    bqk_sb[:, 2 + m : 3 + m],
                )

            # ---- v projection (emitted later for tb0/p0 split; see
            # emit_vproj) ----
            vproj_state = {}

            def emit_xv(kb_i):
                if kb_i not in vproj_state:
                    kbw = kbs[kb_i]
                    xv_t = xs.tile(
                        [128, 6, 512], f32r, tag="xv", name="xv_t",
                        bufs=max(2, len(kbs)),
                    )
                    nc.sync.dma_start(
                        out=xv_t[:, :, :kbw], in_=XV[:, kb_i, :, :kbw]
                    )
                    vproj_state[kb_i] = xv_t
                return vproj_state[kb_i]

            def emit_vproj_kt(kt):
                kb_i = kt // 4
                sub = kt % 4
                xv_t = emit_xv(kb_i)
                ps_t = ps.tile([128, 512], f32, tag="proj", name="vp_ps")
                for k in range(6):
                    nc.tensor.matmul(
                        ps_t[:, :GH],
                        xv_t[:, k, sub * 128 : (sub + 1) * 128],
                        wv_sb[:, k, :],
                        start=(k == 0),
                        stop=(k == 5),
                    )
                nc.vector.tensor_copy(
                    v_sb[:, kt, :, 0:D],
                    ps_t[:, :GH].rearrange("p (h d) -> p h d", h=HL),
                )

            def emit_xq(tb, half=None):
                if half is None:
                    nc.sync.dma_start(out=xq_all[:, tb], in_=XQ[:, tb])
                else:
                    k0, k1 = (0, 4) if half == 0 else (4, 8)
                    nc.sync.dma_start(
                        out=xq_all[:, tb, k0:k1], in_=XQ[:, tb, k0:k1]
                    )

            def emit_qproj_m(tb, m):
                t0 = tb * TB
                ps_t = ps.tile([128, 512], f32, tag="proj", name="qp_ps")
                for k in range(8):
                    nc.tensor.matmul(
                        ps_t,
                        wq_sb[:, k, m * 128 : (m + 1) * 128],
                        xq_all[:, tb, k, :],
                        start=(k == 0),
                        stop=(k == 7),
                    )
                nc.vector.tensor_scalar_add(
                    qT[m][:, t0 : t0 + TB],
                    ps_t,
                    bqk_sb[:, m : m + 1],
                )

            def emit_scores(p, tb, kt, prtag, prbufs):
                t0 = tb * TB
                k0 = kt * 128
                sc = ps.tile([128, 2, TB], f32, tag="sc", name="sc")
                for hh in range(2):
                    nc.tensor.matmul(
                        sc[:, hh, :],
                        kT[p][hh * 64 : hh * 64 + 64, k0 : k0 + 128],
                        qT[p][hh * 64 : hh * 64 + 64, t0 : t0 + TB],
                        start=True,
                        stop=True,
                        tile_position=(hh * 64, 0),
                    )
                pr = probs_pool.tile(
                    [128, 2, TB], f32r, tag=prtag, name="pr", bufs=prbufs
                )
                nc.scalar.activation(
                    pr, sc, Exp, bias=mb_sb[:, kt : kt + 1], scale=0.125
                )
                return pr

            def emit_pv(p, tb, kt, pr, ctx_ps):
                for hh in range(2):
                    nc.tensor.matmul(
                        ctx_ps[hh],
                        v_sb[:, kt, 2 * p + hh, :],
                        pr[:, hh, :],
                        start=(kt == 0),
                        stop=(kt == nkt - 1),
                    )

            def emit_normalize(p, tb, ctx_ps):
                t0 = tb * TB
                # evacuate both PSUM ctx tiles to SBUF first (quick DVE
                # copies) so the next block's PV matmuls get their PSUM
                # slots immediately; the normalize chain then runs from
                # SBUF off the PE critical path
                evac = []
                for hh in range(2):
                    ctmp = norm_pool.tile([D, TB], f32, tag="ctmp", name="ctmp")
                    nc.vector.tensor_copy(ctmp, ctx_ps[hh][0:D, :])
                    dcp = norm_pool.tile([1, TB], f32, tag="dcp", name="dcp")
                    nc.vector.tensor_copy(dcp, ctx_ps[hh][D : D + 1, :])
                    evac.append((ctmp, dcp))
                for hh in range(2):
                    ctmp, dcp = evac[hh]
                    rbc = norm_pool.tile([D, TB], f32, tag="rbc", name="rbc")
                    nc.gpsimd.partition_broadcast(rbc, dcp)
                    rec = norm_pool.tile([D, TB], f32, tag="rec", name="rec")
                    nc.vector.reciprocal_approx_fast(out=rec, in_=rbc)
                    dst = qT[p][hh * 64 : hh * 64 + 64, t0 : t0 + TB]
                    nc.vector.tensor_mul(dst, ctmp, rec)
                    if with_bv:
                        nc.vector.tensor_scalar_add(
                            dst, dst, bv_sb[64 * hh : 64 * hh + 64, p : p + 1]
                        )

            def emit_attn(p, tb, thunks=()):
                # thunk runs between the scores pair and the PV pair of each
                # kt: the interleaved PE work hides the exp(kt) latency the
                # PV would otherwise stall on
                thunks = list(thunks)
                ctx_ps = [
                    ps.tile([D + 1, TB], f32, tag="ctx", name=f"ctx{p}_{tb}_{i}")
                    for i in range(2)
                ]
                for kt in range(nkt):
                    pr = emit_scores(p, tb, kt, "pr", 4)
                    if kt < len(thunks):
                        thunks[kt]()
                    emit_pv(p, tb, kt, pr, ctx_ps)
                for th in thunks[nkt:]:
                    th()
                emit_normalize(p, tb, ctx_ps)

            def qproj_thunks(tb):
                # 6 small thunks: 3+3+2 matmuls per m-half, chained into
                # one psum accumulation (interleaving with other banks is
                # legal), evac on the last
                cell = {}

                def chunk(m, k0, k1, fin, tb=tb):
                    if k0 == 0:
                        cell[m] = ps.tile(
                            [128, 512], f32, tag="proj", name="qp_ps"
                        )
                    ps_t = cell[m]
                    for k in range(k0, k1):
                        nc.tensor.matmul(
                            ps_t,
                            wq_sb[:, k, m * 128 : (m + 1) * 128],
                            xq_all[:, tb, k, :],
                            start=(k == 0),
                            stop=(k == 7),
                        )
                    if fin:
                        nc.vector.tensor_scalar_add(
                            qT[m][:, tb * TB : (tb + 1) * TB],
                            ps_t,
                            bqk_sb[:, m : m + 1],
                        )

                out = []
                for m in range(2):
                    out += [
                        lambda m=m: chunk(m, 0, 3, False),
                        lambda m=m: chunk(m, 3, 6, False),
                        lambda m=m: chunk(m, 6, 8, True),
                    ]
                return out

            def outproj_thunks(tb):
                # four leading no-ops: the first pieces otherwise stall the
                # in-order PE queue on the ~3.5us normalize chain
                # (DVE copy -> gpsimd broadcast -> reciprocal -> mul) of
                # the block whose ctx they consume, starving ACT
                out = [lambda: None] * 4
                for tt in range(4 * tb, 4 * tb + 4):
                    for nh in range(2):
                        out.append(
                            lambda tt=tt, nh=nh: emit_outproj_nh(tt, nh)
                        )
                return out

            def emit_outproj_nh(tt, nh):
                # evac on DVE (keeps ACT free for the exps) and DMA on the
                # vector HWDGE ring so the sync ring stays dedicated to the
                # big input streams
                ps_t = ps.tile([128, 512], f32, tag="proj", name="op_ps")
                for kk in range(2):
                    nc.tensor.matmul(
                        ps_t,
                        qT[kk][:, tt * 128 : (tt + 1) * 128],
                        wo_sb[:, kk, nh * 512 : (nh + 1) * 512],
                        start=(kk == 0),
                        stop=(kk == 1),
                    )
                o_sb = outs_pool.tile(
                    [128, 512], f32r, tag="osb", name="o_sb"
                )
                nc.vector.tensor_copy(o_sb, ps_t)
                nc.sync.dma_start(out=OUT[tt, nh], in_=o_sb)

            def emit_outproj(tb):
                for tt in range(4 * tb, 4 * tb + 4):
                    for nh in range(2):
                        emit_outproj_nh(tt, nh)

            # ---- emission schedule ----
            # DMA priority order on the sync ring: xk0, xq0, xk1.., xv
            # blocks, then the remaining xq tiles — everything is issued
            # up front so out-DMAs appended later never delay inputs.
            # Preamble: minimal PE path to the first exp (kb0/m0 + q/m0 +
            # scores kt0..3); steady state: every attention group carries
            # PE filler thunks (qproj of tb+1 or outproj of tb-1) so the
            # PE never runs a matmul-only phase while ACT idles
            xk_ts = [emit_xk(0)]
            nc.sync.dma_start(out=wq_sb, in_=WQ)
            emit_xq(0, half=0)
            emit_xq(0, half=1)
            for kb_i in range(1, len(kbs)):
                xk_ts.append(emit_xk(kb_i))
            for kb_i in range(len(kbs)):
                emit_xv(kb_i)
            for tb in range(1, NTB):
                emit_xq(tb)
            emit_kproj_m(0, 0, xk_ts[0])
            emit_qproj_m(0, 0)
            kt_kb0 = min(4, nkt)
            prs0 = [emit_scores(0, 0, kt, "pr0", nkt) for kt in range(kt_kb0)]
            emit_kproj_m(0, 1, xk_ts[0])
            emit_qproj_m(0, 1)
            for kb_i in range(1, len(kbs)):
                emit_kproj_m(kb_i, 0, xk_ts[kb_i])
                emit_kproj_m(kb_i, 1, xk_ts[kb_i])
            prs0 += [
                emit_scores(0, 0, kt, "pr0", nkt) for kt in range(kt_kb0, nkt)
            ]
            for kt in range(nkt):
                emit_vproj_kt(kt)
            ctx0 = [
                ps.tile([D + 1, TB], f32, tag="ctx", name=f"ctx00_{i}")
                for i in range(2)
            ]
            for kt in range(nkt):
                emit_pv(0, 0, kt, prs0[kt], ctx0)
            emit_normalize(0, 0, ctx0)
            emit_attn(1, 0, thunks=qproj_thunks(1))
            for tb in range(1, NTB):
                emit_attn(0, tb, thunks=outproj_thunks(tb - 1))
                if tb < NTB - 1:
                    emit_attn(1, tb, thunks=qproj_thunks(tb + 1))
                else:
                    emit_attn(1, tb)
            emit_outproj(NTB - 1)

    nc.compile()
    return nc


def kernel(
    query, key, value, Wq, bq, Wk, bk, Wv, bv, Wo, bo, query_mask, key_mask
):
    global LAST_EXEC_NS, LAST_TRACE_DIR
    from concourse.bass_utils import run_bass_kernel_spmd

    query = np.asarray(query, dtype=np.float32)
    key = np.asarray(key, dtype=np.float32)
    value = np.asarray(value, dtype=np.float32)
    Wq = np.asarray(Wq, dtype=np.float32)
    Wk = np.asarray(Wk, dtype=np.float32)
    Wv = np.asarray(Wv, dtype=np.float32)
    Wo = np.asarray(Wo, dtype=np.float32)
    bq = np.asarray(bq, dtype=np.float32)
    bk = np.asarray(bk, dtype=np.float32)
    bv = np.asarray(bv, dtype=np.float32)
    bo = np.asarray(bo, dtype=np.float32)
    qm = np.asarray(query_mask)
    km = np.asarray(key_mask)

    # host-side key compaction (query_mask masks the KEY axis, globally
    # per batch)
    keep = [np.flatnonzero(qm[b] != 0) for b in range(B)]
    nkeep = max((len(k) for k in keep), default=0)
    nkt = max(1, math.ceil(nkeep / 128))
    nkeys = nkt * 128

    with_bv = bool(np.any(bv))
    ck = (nkt, with_bv, BF16)
    if ck not in _CACHE:
        _CACHE[ck] = _build(nkt, with_bv, BF16)
    nc = _CACHE[ck]

    wdt = ml_dtypes.bfloat16 if BF16 else np.float32
    nkb = math.ceil(nkeys / 512)
    nkeys_b = nkb * 512

    def arr_kmajor(a, ktiles):  # [dim, n] -> [128, ktiles, n]
        return np.ascontiguousarray(
            a.reshape(ktiles, 128, a.shape[1]).transpose(1, 0, 2)
        ).astype(wdt)

    def arr_blocked(a, ktiles, blocks):  # [dim, n] -> [128, blocks, kt, 512]
        return np.ascontiguousarray(
            a.reshape(ktiles, 128, blocks, 512).transpose(1, 2, 0, 3)
        ).astype(wdt)

    in_maps = []
    for c in range(N_CORES):
        b, hg = c // HG, c % HG
        hs = hg * GH
        idx = keep[b]
        # compacted + padded key/value (transposed)
        xk = np.zeros((KDIM, nkeys_b), np.float32)
        xk[:, : len(idx)] = key[b].T[:, idx]
        xv = np.zeros((VDIM, nkeys_b), np.float32)
        xv[:, : len(idx)] = value[b].T[:, idx]
        mbias = np.full((nkeys,), NEG, np.float32)
        mbias[: len(idx)] = 0.0
        bqk = np.empty((128, 4), np.float32)
        bqk[:, 0] = bq[hs : hs + 128]
        bqk[:, 1] = bq[hs + 128 : hs + 256]
        bqk[:, 2] = bk[hs : hs + 128]
        bqk[:, 3] = bk[hs + 128 : hs + 256]
        m = {
            "xq": arr_blocked(query[b].T, 8, NTB),
            "xk": arr_blocked(xk, 6, nkb),
            "xv": arr_blocked(xv, 6, nkb),
            "wq": arr_kmajor(Wq[:, hs : hs + GH], 8),
            "wk": arr_kmajor(Wk[:, hs : hs + GH], 6),
            "wv": arr_kmajor(Wv[:, hs : hs + GH], 6),
            "wo": arr_kmajor(Wo[hs : hs + GH, :], 2),
            "mbias": np.ascontiguousarray(mbias.reshape(nkt, 128).T),
            "bqk": bqk,
        }
        if with_bv:
            bvt = np.empty((128, 2), np.float32)
            bvt[:, 0] = bv[hs : hs + 128]
            bvt[:, 1] = bv[hs + 128 : hs + 256]
            m["bv"] = bvt
        in_maps.append(m)

    kwargs = {}
    if PROFILE:
        import tempfile

        LAST_TRACE_DIR = tempfile.mkdtemp(prefix="bass_trace_")
        kwargs = {"trace": True, "tmpdir": LAST_TRACE_DIR}
    res = run_bass_kernel_spmd(nc, in_maps, list(range(N_CORES)), **kwargs)
    LAST_EXEC_NS = res.exec_time_ns

    out = np.zeros((B, LQ, QDIM), np.float32)
    for c in range(N_CORES):
        blk = res.results[c]["outp"]  # [LQ//128, 2, 128, 512] block-major
        out[c // HG] += blk.transpose(0, 2, 1, 3).reshape(LQ, QDIM)
    out += bo[None, None, :]
    for b in range(B):
        if len(keep[b]) == 0:
            # all keys masked: reference softmax is NaN everywhere
            out[b] = np.nan
    # key_mask masks the QUERY axis in the reference; a zero row makes the
    # whole softmax row -inf -> NaN output for that query position.
    for b in range(B):
        zq = np.flatnonzero(km[b] == 0)
        if len(zq):
            out[b, zq, :] = np.nan
    return out



# revision 33
# speedup vs baseline: 1.1301x; 1.0141x over previous
"""Trainium2 Bass kernel for nn_CrossModalAttention (B=2, LQ=LK=2048,
QDIM=HID=1024, KDIM=VDIM=768, H=16, D=64).

Sharding: 8 cores = 2 batches x 4 head-groups (4 heads each).
Per core: q/k/v projections column-sliced over HID, attention for its 4
heads, row-parallel partial of the out-projection. Host sums the 4
partials per batch (the row-parallel unshard) and adds bo.

Device dataflow (per core), all matmuls in fp32r (TF32-like, ~1.5e-4):
  - host passes query/key/value[b] transposed (and K/V key-compacted:
    query_mask masks the KEY axis globally per batch, so masked keys are
    dropped on host and the remainder padded to a multiple of 128)
  - qT/kT [hid, tokens] and v [keys, hid] computed on device
  - per head pair (row-packed K=64 matmuls via tile_position):
    scoresT [keys, q] -> ACT exp(s/8 + mask_bias) -> PV matmul with a
    ones-augmented V (M=65) giving ctxT and the softmax denominator
  - normalize on DVE (reciprocal + gpsimd partition-broadcast)
  - out-projection from ctxT, partial written to DRAM
"""

import math

import ml_dtypes
import numpy as np

B, LQ, LK = 2, 2048, 2048
QDIM, KDIM, VDIM, HID, H = 1024, 768, 768, 1024, 16
D = HID // H  # 64
HG = 4  # head-groups (cores per batch)
HL = H // HG  # heads per core = 4
GH = HL * D  # per-core hid slice = 256
N_CORES = 8
TB = 512  # token block
NTB = LQ // TB  # 4
NEG = -1.0e30

BF16 = True
PROFILE = False
LAST_EXEC_NS = None
LAST_TRACE_DIR = None

_CACHE = {}


def _build(nkt: int, with_bv: bool, bf16: bool):
    import concourse.bacc as bacc
    import concourse.mybir as mybir
    import concourse.tile as tile

    nkeys = nkt * 128
    # key blocks of <=512 for the k-projection
    kbs = [min(512, nkeys - s) for s in range(0, nkeys, 512)]

    f32 = mybir.dt.float32
    f32r = mybir.dt.bfloat16 if bf16 else mybir.dt.float32r
    Exp = mybir.ActivationFunctionType.Exp
    Ident = mybir.ActivationFunctionType.Identity

    nc = bacc.Bacc(
        "TRN2", target_bir_lowering=False, debug=False, num_devices=N_CORES
    )

    nkb = len(kbs)
    # DRAM tensors (per-core shapes). Inputs are laid out block-major so
    # every DMA moves large contiguous per-partition lines (1KB strided
    # lines are descriptor-rate-bound: ~15us for 1MB)
    XQ = nc.dram_tensor(
        "xq", [128, NTB, 8, TB], f32r, kind="ExternalInput"
    ).ap()
    XK = nc.dram_tensor(
        "xk", [128, nkb, 6, 512], f32r, kind="ExternalInput"
    ).ap()
    XV = nc.dram_tensor(
        "xv", [128, nkb, 6, 512], f32r, kind="ExternalInput"
    ).ap()
    WQ = nc.dram_tensor("wq", [128, 8, GH], f32r, kind="ExternalInput").ap()
    WK = nc.dram_tensor("wk", [128, 6, GH], f32r, kind="ExternalInput").ap()
    WV = nc.dram_tensor("wv", [128, 6, GH], f32r, kind="ExternalInput").ap()
    WO = nc.dram_tensor("wo", [128, 2, QDIM], f32r, kind="ExternalInput").ap()
    MB = nc.dram_tensor("mbias", [128, nkt], f32, kind="ExternalInput").ap()
    BQ = nc.dram_tensor("bqk", [128, 4], f32, kind="ExternalInput").ap()
    BV = None
    if with_bv:
        BV = nc.dram_tensor("bv", [128, 2], f32, kind="ExternalInput").ap()
    # output block-major: [tt, nh, 128, 512] so each store is one fully
    # contiguous write; bf16 halves the drain (host sums partials in f32)
    OUT = nc.dram_tensor(
        "outp", [LQ // 128, 2, 128, 512], f32r, kind="ExternalOutput"
    ).ap()

    with tile.TileContext(nc) as tc:
        with (
            tc.tile_pool(name="consts", bufs=1) as consts,
            tc.tile_pool(name="resid", bufs=1) as resid,
            tc.tile_pool(name="xs", bufs=2) as xs,
            tc.tile_pool(name="probs", bufs=4) as probs_pool,
            tc.tile_pool(name="norm", bufs=3) as norm_pool,
            tc.tile_pool(name="outs", bufs=3) as outs_pool,
            tc.tile_pool(name="ps", bufs=2, space="PSUM") as ps,
        ):
            # ---- constants / weights ----
            # wk/wq gate the very first matmuls: they go FIRST on the sync
            # HWDGE ring (the gpsimd SWDGE ring takes ~7us to complete a
            # transfer, which used to stall the PE until ~15us). wv/wo are
            # needed later and stay on the SWDGE ring.
            wq_sb = consts.tile([128, 8, GH], f32r)
            wk_sb = consts.tile([128, 6, GH], f32r)
            wv_sb = consts.tile([128, 6, GH], f32r)
            wo_sb = consts.tile([128, 2, QDIM], f32r)
            mb_sb = consts.tile([128, nkt], f32)
            bqk_sb = consts.tile([128, 4], f32)
            nc.sync.dma_start(out=wk_sb, in_=WK)
            nc.scalar.dma_start(out=bqk_sb, in_=BQ)
            nc.scalar.dma_start(out=mb_sb, in_=MB)
            nc.gpsimd.dma_start(out=wv_sb, in_=WV)
            nc.gpsimd.dma_start(out=wo_sb, in_=WO)
            bv_sb = None
            if with_bv:
                bv_sb = consts.tile([128, 2], f32)
                nc.gpsimd.dma_start(out=bv_sb, in_=BV)

            # ---- residents ----
            # qT tiles double as ctxT tiles later (WAR handled by Tile)
            qT = [resid.tile([128, LQ], f32r, tag=f"qT{p}", name=f"qT{p}") for p in range(2)]
            kT = [resid.tile([128, nkeys], f32r, tag=f"kT{p}", name=f"kT{p}") for p in range(2)]
            xq_all = resid.tile([128, NTB, 8, TB], f32r, tag="xq", name="xq")
            v_sb = resid.tile([128, nkt, HL, D + 1], f32r)
            # ones columns for the denominator rows: fill the whole tile,
            # the v-projection copies then overwrite the [., ., ., 0:D] part
            if bf16:
                nc.vector.memset(v_sb, 1.0)
            else:
                nc.vector.memset(v_sb[:, :, :, :].bitcast(f32), 1.0)

            # ---- k projection (per key-block, so attention can begin
            # after kb0) ----
            def emit_xk(kb_i):
                kbw = kbs[kb_i]
                xk_t = xs.tile([128, 6, 512], f32r, tag="xk", name="xk_t", bufs=3)
                nc.sync.dma_start(
                    out=xk_t[:, :, :kbw], in_=XK[:, kb_i, :, :kbw]
                )
                return xk_t

            def emit_kproj_m(kb_i, m, xk_t):
                kbw = kbs[kb_i]
                s0 = kb_i * 512
                ps_t = ps.tile([128, 512], f32, tag="proj", name="kp_ps")
                for k in range(6):
                    nc.tensor.matmul(
                        ps_t[:, :kbw],
                        wk_sb[:, k, m * 128 : (m + 1) * 128],
                        xk_t[:, k, :kbw],
                        start=(k == 0),
                        stop=(k == 5),
                    )
                nc.vector.tensor_scalar_add(
                    kT[m][:, s0 : s0 + kbw],
                    ps_t[:, :kbw],
                    bqk_sb[:, 2 + m : 3 + m],
                )

            # ---- v projection (emitted later for tb0/p0 split; see
            # emit_vproj) ----
            vproj_state = {}

            def emit_xv(kb_i):
                if kb_i not in vproj_state:
                    kbw = kbs[kb_i]
                    xv_t = xs.tile(
                        [128, 6, 512], f32r, tag="xv", name="xv_t",
                        bufs=max(2, len(kbs)),
                    )
                    nc.sync.dma_start(
                        out=xv_t[:, :, :kbw], in_=XV[:, kb_i, :, :kbw]
                    )
                    vproj_state[kb_i] = xv_t
                return vproj_state[kb_i]

            def emit_vproj_kt(kt):
                kb_i = kt // 4
                sub = kt % 4
                xv_t = emit_xv(kb_i)
                ps_t = ps.tile([128, 512], f32, tag="proj", name="vp_ps")
                for k in range(6):
                    nc.tensor.matmul(
                        ps_t[:, :GH],
                        xv_t[:, k, sub * 128 : (sub + 1) * 128],
                        wv_sb[:, k, :],
                        start=(k == 0),
                        stop=(k == 5),
                    )
                nc.vector.tensor_copy(
                    v_sb[:, kt, :, 0:D],
                    ps_t[:, :GH].rearrange("p (h d) -> p h d", h=HL),
                )

            def emit_xq(tb, half=None):
                if half is None:
                    nc.sync.dma_start(out=xq_all[:, tb], in_=XQ[:, tb])
                else:
                    k0, k1 = (0, 4) if half == 0 else (4, 8)
                    nc.sync.dma_start(
                        out=xq_all[:, tb, k0:k1], in_=XQ[:, tb, k0:k1]
                    )

            def emit_qproj_m(tb, m):
                t0 = tb * TB
                ps_t = ps.tile([128, 512], f32, tag="proj", name="qp_ps")
                for k in range(8):
                    nc.tensor.matmul(
                        ps_t,
                        wq_sb[:, k, m * 128 : (m + 1) * 128],
                        xq_all[:, tb, k, :],
                        start=(k == 0),
                        stop=(k == 7),
                    )
                nc.vector.tensor_scalar_add(
                    qT[m][:, t0 : t0 + TB],
                    ps_t,
                    bqk_sb[:, m : m + 1],
                )

            def emit_scores(p, tb, kt, prtag, prbufs):
                t0 = tb * TB
                k0 = kt * 128
                sc = ps.tile([128, 2, TB], f32, tag="sc", name="sc")
                for hh in range(2):
                    nc.tensor.matmul(
                        sc[:, hh, :],
                        kT[p][hh * 64 : hh * 64 + 64, k0 : k0 + 128],
                        qT[p][hh * 64 : hh * 64 + 64, t0 : t0 + TB],
                        start=True,
                        stop=True,
                        tile_position=(hh * 64, 0),
                    )
                pr = probs_pool.tile(
                    [128, 2, TB], f32r, tag=prtag, name="pr", bufs=prbufs
                )
                nc.scalar.activation(
                    pr, sc, Exp, bias=mb_sb[:, kt : kt + 1], scale=0.125
                )
                return pr

            def emit_pv(p, tb, kt, pr, ctx_ps):
                for hh in range(2):
                    nc.tensor.matmul(
                        ctx_ps[hh],
                        v_sb[:, kt, 2 * p + hh, :],
                        pr[:, hh, :],
                        start=(kt == 0),
                        stop=(kt == nkt - 1),
                    )

            def emit_normalize(p, tb, ctx_ps):
                t0 = tb * TB
                # evacuate both PSUM ctx tiles to SBUF first (quick DVE
                # copies) so the next block's PV matmuls get their PSUM
                # slots immediately; the normalize chain then runs from
                # SBUF off the PE critical path
                evac = []
                for hh in range(2):
                    ctmp = norm_pool.tile([D, TB], f32, tag="ctmp", name="ctmp")
                    nc.vector.tensor_copy(ctmp, ctx_ps[hh][0:D, :])
                    dcp = norm_pool.tile([1, TB], f32, tag="dcp", name="dcp")
                    nc.vector.tensor_copy(dcp, ctx_ps[hh][D : D + 1, :])
                    evac.append((ctmp, dcp))
                for hh in range(2):
                    ctmp, dcp = evac[hh]
                    rbc = norm_pool.tile([D, TB], f32, tag="rbc", name="rbc")
                    nc.gpsimd.partition_broadcast(rbc, dcp)
                    rec = norm_pool.tile([D, TB], f32, tag="rec", name="rec")
                    nc.vector.reciprocal_approx_fast(out=rec, in_=rbc)
                    dst = qT[p][hh * 64 : hh * 64 + 64, t0 : t0 + TB]
                    nc.vector.tensor_mul(dst, ctmp, rec)
                    if with_bv:
                        nc.vector.tensor_scalar_add(
                            dst, dst, bv_sb[64 * hh : 64 * hh + 64, p : p + 1]
                        )

            def emit_attn(p, tb, thunks=(), prs=None):
                # kt processed in pairs: adjacent score pairs keep their
                # LDWEIGHTS together (interleaved full-height thunk loads
                # otherwise split the row-tiled pair concurrency) and the
                # exp of each kt gets a full pair-cycle of lead time
                # before its PV consumes it
                thunks = list(thunks)
                ti = 0

                def run_thunk():
                    nonlocal ti
                    if ti < len(thunks):
                        thunks[ti]()
                        ti += 1

                ctx_ps = [
                    ps.tile([D + 1, TB], f32, tag="ctx", name=f"ctx{p}_{tb}_{i}")
                    for i in range(2)
                ]
                for kt0 in range(0, nkt, 2):
                    kts = [kt0] + ([kt0 + 1] if kt0 + 1 < nkt else [])
                    prl = []
                    for kt in kts:
                        prl.append(
                            prs[kt] if prs is not None
                            else emit_scores(p, tb, kt, "pr", 4)
                        )
                    run_thunk()
                    for i, kt in enumerate(kts):
                        emit_pv(p, tb, kt, prl[i], ctx_ps)
                    run_thunk()
                while ti < len(thunks):
                    thunks[ti]()
                    ti += 1
                emit_normalize(p, tb, ctx_ps)

            def qproj_thunks(tb):
                # 6 small thunks: 3+3+2 matmuls per m-half, chained into
                # one psum accumulation (interleaving with other banks is
                # legal), evac on the last
                cell = {}

                def chunk(m, k0, k1, fin, tb=tb):
                    if k0 == 0:
                        cell[m] = ps.tile(
                            [128, 512], f32, tag="proj", name="qp_ps"
                        )
                    ps_t = cell[m]
                    for k in range(k0, k1):
                        nc.tensor.matmul(
                            ps_t,
                            wq_sb[:, k, m * 128 : (m + 1) * 128],
                            xq_all[:, tb, k, :],
                            start=(k == 0),
                            stop=(k == 7),
                        )
                    if fin:
                        nc.vector.tensor_scalar_add(
                            qT[m][:, tb * TB : (tb + 1) * TB],
                            ps_t,
                            bqk_sb[:, m : m + 1],
                        )

                out = []
                for m in range(2):
                    out += [
                        lambda m=m: chunk(m, 0, 3, False),
                        lambda m=m: chunk(m, 3, 6, False),
                        lambda m=m: chunk(m, 6, 8, True),
                    ]
                return out

            def outproj_thunks(tb):
                # two leading no-ops: the first pieces otherwise stall the
                # in-order PE queue on the ~3.5us normalize chain
                # (DVE copy -> gpsimd broadcast -> reciprocal -> mul) of
                # the block whose ctx they consume, starving ACT
                out = [lambda: None] * 2
                for tt in range(4 * tb, 4 * tb + 4):
                    for nh in range(2):
                        out.append(
                            lambda tt=tt, nh=nh: emit_outproj_nh(tt, nh)
                        )
                return out

            def emit_outproj_nh(tt, nh):
                # evac on DVE (keeps ACT free for the exps) and DMA on the
                # vector HWDGE ring so the sync ring stays dedicated to the
                # big input streams
                ps_t = ps.tile([128, 512], f32, tag="proj", name="op_ps")
                for kk in range(2):
                    nc.tensor.matmul(
                        ps_t,
                        qT[kk][:, tt * 128 : (tt + 1) * 128],
                        wo_sb[:, kk, nh * 512 : (nh + 1) * 512],
                        start=(kk == 0),
                        stop=(kk == 1),
                    )
                o_sb = outs_pool.tile(
                    [128, 512], f32r, tag="osb", name="o_sb"
                )
                nc.vector.tensor_copy(o_sb, ps_t)
                nc.sync.dma_start(out=OUT[tt, nh], in_=o_sb)

            def emit_outproj(tb):
                for tt in range(4 * tb, 4 * tb + 4):
                    for nh in range(2):
                        emit_outproj_nh(tt, nh)

            # ---- emission schedule ----
            # DMA priority order on the sync ring: xk0, xq0, xk1.., xv
            # blocks, then the remaining xq tiles — everything is issued
            # up front so out-DMAs appended later never delay inputs.
            # Preamble: minimal PE path to the first exp (kb0/m0 + q/m0 +
            # scores kt0..3); steady state: every attention group carries
            # PE filler thunks (qproj of tb+1 or outproj of tb-1) so the
            # PE never runs a matmul-only phase while ACT idles
            xk_ts = [emit_xk(0)]
            nc.sync.dma_start(out=wq_sb, in_=WQ)
            emit_xq(0, half=0)
            emit_xq(0, half=1)
            for kb_i in range(1, len(kbs)):
                xk_ts.append(emit_xk(kb_i))
            for kb_i in range(len(kbs)):
                emit_xv(kb_i)
            for tb in range(1, NTB):
                emit_xq(tb)
            emit_kproj_m(0, 0, xk_ts[0])
            emit_qproj_m(0, 0)
            kt_kb0 = min(4, nkt)
            prs0 = [emit_scores(0, 0, kt, "pr0", nkt) for kt in range(kt_kb0)]
            emit_kproj_m(0, 1, xk_ts[0])
            emit_qproj_m(0, 1)
            for kb_i in range(1, len(kbs)):
                emit_kproj_m(kb_i, 0, xk_ts[kb_i])
                emit_kproj_m(kb_i, 1, xk_ts[kb_i])
            prs0 += [
                emit_scores(0, 0, kt, "pr0", nkt) for kt in range(kt_kb0, nkt)
            ]
            # p=1 scores issued up front too: their exps keep ACT busy
            # through the (otherwise matmul-only) vproj + first-PV phase
            prs1 = [emit_scores(1, 0, kt, "pr1", nkt) for kt in range(nkt)]
            for kt in range(nkt):
                emit_vproj_kt(kt)
            emit_attn(0, 0, prs=prs0)
            emit_attn(1, 0, thunks=qproj_thunks(1), prs=prs1)
            for tb in range(1, NTB):
                emit_attn(0, tb, thunks=outproj_thunks(tb - 1))
                if tb < NTB - 1:
                    emit_attn(1, tb, thunks=qproj_thunks(tb + 1))
                else:
                    emit_attn(1, tb)
            emit_outproj(NTB - 1)

    nc.compile()
    return nc


def kernel(
    query, key, value, Wq, bq, Wk, bk, Wv, bv, Wo, bo, query_mask, key_mask
):
    global LAST_EXEC_NS, LAST_TRACE_DIR
    from concourse.bass_utils import run_bass_kernel_spmd

    query = np.asarray(query, dtype=np.float32)
    key = np.asarray(key, dtype=np.float32)
    value = np.asarray(value, dtype=np.float32)
    Wq = np.asarray(Wq, dtype=np.float32)
    Wk = np.asarray(Wk, dtype=np.float32)
    Wv = np.asarray(Wv, dtype=np.float32)
    Wo = np.asarray(Wo, dtype=np.float32)
    bq = np.asarray(bq, dtype=np.float32)
    bk = np.asarray(bk, dtype=np.float32)
    bv = np.asarray(bv, dtype=np.float32)
    bo = np.asarray(bo, dtype=np.float32)
    qm = np.asarray(query_mask)
    km = np.asarray(key_mask)

    # host-side key compaction (query_mask masks the KEY axis, globally
    # per batch)
    keep = [np.flatnonzero(qm[b] != 0) for b in range(B)]
    nkeep = max((len(k) for k in keep), default=0)
    nkt = max(1, math.ceil(nkeep / 128))
    nkeys = nkt * 128

    with_bv = bool(np.any(bv))
    ck = (nkt, with_bv, BF16)
    if ck not in _CACHE:
        _CACHE[ck] = _build(nkt, with_bv, BF16)
    nc = _CACHE[ck]

    wdt = ml_dtypes.bfloat16 if BF16 else np.float32
    nkb = math.ceil(nkeys / 512)
    nkeys_b = nkb * 512

    def arr_kmajor(a, ktiles):  # [dim, n] -> [128, ktiles, n]
        return np.ascontiguousarray(
            a.reshape(ktiles, 128, a.shape[1]).transpose(1, 0, 2)
        ).astype(wdt)

    def arr_blocked(a, ktiles, blocks):  # [dim, n] -> [128, blocks, kt, 512]
        return np.ascontiguousarray(
            a.reshape(ktiles, 128, blocks, 512).transpose(1, 2, 0, 3)
        ).astype(wdt)

    in_maps = []
    for c in range(N_CORES):
        b, hg = c // HG, c % HG
        hs = hg * GH
        idx = keep[b]
        # compacted + padded key/value (transposed)
        xk = np.zeros((KDIM, nkeys_b), np.float32)
        xk[:, : len(idx)] = key[b].T[:, idx]
        xv = np.zeros((VDIM, nkeys_b), np.float32)
        xv[:, : len(idx)] = value[b].T[:, idx]
        mbias = np.full((nkeys,), NEG, np.float32)
        mbias[: len(idx)] = 0.0
        bqk = np.empty((128, 4), np.float32)
        bqk[:, 0] = bq[hs : hs + 128]
        bqk[:, 1] = bq[hs + 128 : hs + 256]
        bqk[:, 2] = bk[hs : hs + 128]
        bqk[:, 3] = bk[hs + 128 : hs + 256]
        m = {
            "xq": arr_blocked(query[b].T, 8, NTB),
            "xk": arr_blocked(xk, 6, nkb),
            "xv": arr_blocked(xv, 6, nkb),
            "wq": arr_kmajor(Wq[:, hs : hs + GH], 8),
            "wk": arr_kmajor(Wk[:, hs : hs + GH], 6),
            "wv": arr_kmajor(Wv[:, hs : hs + GH], 6),
            "wo": arr_kmajor(Wo[hs : hs + GH, :], 2),
            "mbias": np.ascontiguousarray(mbias.reshape(nkt, 128).T),
            "bqk": bqk,
        }
        if with_bv:
            bvt = np.empty((128, 2), np.float32)
            bvt[:, 0] = bv[hs : hs + 128]
            bvt[:, 1] = bv[hs + 128 : hs + 256]
            m["bv"] = bvt
        in_maps.append(m)

    kwargs = {}
    if PROFILE:
        import tempfile

        LAST_TRACE_DIR = tempfile.mkdtemp(prefix="bass_trace_")
        kwargs = {"trace": True, "tmpdir": LAST_TRACE_DIR}
    res = run_bass_kernel_spmd(nc, in_maps, list(range(N_CORES)), **kwargs)
    LAST_EXEC_NS = res.exec_time_ns

    out = np.zeros((B, LQ, QDIM), np.float32)
    for c in range(N_CORES):
        blk = res.results[c]["outp"]  # [LQ//128, 2, 128, 512] block-major
        out[c // HG] += blk.transpose(0, 2, 1, 3).reshape(LQ, QDIM)
    out += bo[None, None, :]
    for b in range(B):
        if len(keep[b]) == 0:
            # all keys masked: reference softmax is NaN everywhere
            out[b] = np.nan
    # key_mask masks the QUERY axis in the reference; a zero row makes the
    # whole softmax row -inf -> NaN output for that query position.
    for b in range(B):
        zq = np.flatnonzero(km[b] == 0)
        if len(zq):
            out[b, zq, :] = np.nan
    return out



# revision 35
# speedup vs baseline: 1.1373x; 1.0063x over previous
"""Trainium2 Bass kernel for nn_CrossModalAttention (B=2, LQ=LK=2048,
QDIM=HID=1024, KDIM=VDIM=768, H=16, D=64).

Sharding: 8 cores = 2 batches x 4 head-groups (4 heads each).
Per core: q/k/v projections column-sliced over HID, attention for its 4
heads, row-parallel partial of the out-projection. Host sums the 4
partials per batch (the row-parallel unshard) and adds bo.

Device dataflow (per core), all matmuls in fp32r (TF32-like, ~1.5e-4):
  - host passes query/key/value[b] transposed (and K/V key-compacted:
    query_mask masks the KEY axis globally per batch, so masked keys are
    dropped on host and the remainder padded to a multiple of 128)
  - qT/kT [hid, tokens] and v [keys, hid] computed on device
  - per head pair (row-packed K=64 matmuls via tile_position):
    scoresT [keys, q] -> ACT exp(s/8 + mask_bias) -> PV matmul with a
    ones-augmented V (M=65) giving ctxT and the softmax denominator
  - normalize on DVE (reciprocal + gpsimd partition-broadcast)
  - out-projection from ctxT, partial written to DRAM
"""

import math

import ml_dtypes
import numpy as np

B, LQ, LK = 2, 2048, 2048
QDIM, KDIM, VDIM, HID, H = 1024, 768, 768, 1024, 16
D = HID // H  # 64
HG = 4  # head-groups (cores per batch)
HL = H // HG  # heads per core = 4
GH = HL * D  # per-core hid slice = 256
N_CORES = 8
TB = 512  # token block
NTB = LQ // TB  # 4
NEG = -1.0e30

BF16 = True
PROFILE = False
LAST_EXEC_NS = None
LAST_TRACE_DIR = None

_CACHE = {}


def _build(nkt: int, with_bv: bool, bf16: bool):
    import concourse.bacc as bacc
    import concourse.mybir as mybir
    import concourse.tile as tile

    nkeys = nkt * 128
    # key blocks of <=512 for the k-projection
    kbs = [min(512, nkeys - s) for s in range(0, nkeys, 512)]

    f32 = mybir.dt.float32
    f32r = mybir.dt.bfloat16 if bf16 else mybir.dt.float32r
    Exp = mybir.ActivationFunctionType.Exp
    Ident = mybir.ActivationFunctionType.Identity

    nc = bacc.Bacc(
        "TRN2", target_bir_lowering=False, debug=False, num_devices=N_CORES
    )

    nkb = len(kbs)
    # DRAM tensors (per-core shapes). Inputs are laid out block-major so
    # every DMA moves large contiguous per-partition lines (1KB strided
    # lines are descriptor-rate-bound: ~15us for 1MB)
    XQ = nc.dram_tensor(
        "xq", [128, NTB, 8, TB], f32r, kind="ExternalInput"
    ).ap()
    XK = nc.dram_tensor(
        "xk", [128, nkb, 6, 512], f32r, kind="ExternalInput"
    ).ap()
    XV = nc.dram_tensor(
        "xv", [128, nkb, 6, 512], f32r, kind="ExternalInput"
    ).ap()
    WQ = nc.dram_tensor("wq", [128, 8, GH], f32r, kind="ExternalInput").ap()
    WK = nc.dram_tensor("wk", [128, 6, GH], f32r, kind="ExternalInput").ap()
    WV = nc.dram_tensor("wv", [128, 6, GH], f32r, kind="ExternalInput").ap()
    WO = nc.dram_tensor("wo", [128, 2, QDIM], f32r, kind="ExternalInput").ap()
    MB = nc.dram_tensor("mbias", [128, nkt], f32, kind="ExternalInput").ap()
    BQ = nc.dram_tensor("bqk", [128, 4], f32, kind="ExternalInput").ap()
    BV = None
    if with_bv:
        BV = nc.dram_tensor("bv", [128, 2], f32, kind="ExternalInput").ap()
    # output block-major: [tt, nh, 128, 512] so each store is one fully
    # contiguous write; bf16 halves the drain (host sums partials in f32)
    OUT = nc.dram_tensor(
        "outp", [LQ // 128, 2, 128, 512], f32r, kind="ExternalOutput"
    ).ap()

    with tile.TileContext(nc) as tc:
        with (
            tc.tile_pool(name="consts", bufs=1) as consts,
            tc.tile_pool(name="resid", bufs=1) as resid,
            tc.tile_pool(name="xs", bufs=2) as xs,
            tc.tile_pool(name="probs", bufs=4) as probs_pool,
            tc.tile_pool(name="norm", bufs=3) as norm_pool,
            tc.tile_pool(name="outs", bufs=3) as outs_pool,
            tc.tile_pool(name="ps", bufs=2, space="PSUM") as ps,
        ):
            # ---- constants / weights ----
            # wk/wq gate the very first matmuls: they go FIRST on the sync
            # HWDGE ring (the gpsimd SWDGE ring takes ~7us to complete a
            # transfer, which used to stall the PE until ~15us). wv/wo are
            # needed later and stay on the SWDGE ring.
            wq_sb = consts.tile([128, 8, GH], f32r)
            wk_sb = consts.tile([128, 6, GH], f32r)
            wv_sb = consts.tile([128, 6, GH], f32r)
            wo_sb = consts.tile([128, 2, QDIM], f32r)
            mb_sb = consts.tile([128, nkt], f32)
            bqk_sb = consts.tile([128, 4], f32)
            nc.sync.dma_start(out=wk_sb, in_=WK)
            nc.scalar.dma_start(out=bqk_sb, in_=BQ)
            nc.scalar.dma_start(out=mb_sb, in_=MB)
            nc.gpsimd.dma_start(out=wv_sb, in_=WV)
            nc.gpsimd.dma_start(out=wo_sb, in_=WO)
            bv_sb = None
            if with_bv:
                bv_sb = consts.tile([128, 2], f32)
                nc.gpsimd.dma_start(out=bv_sb, in_=BV)

            # ---- residents ----
            # qT tiles double as ctxT tiles later (WAR handled by Tile)
            qT = [resid.tile([128, LQ], f32r, tag=f"qT{p}", name=f"qT{p}") for p in range(2)]
            kT = [resid.tile([128, nkeys], f32r, tag=f"kT{p}", name=f"kT{p}") for p in range(2)]
            xq_all = resid.tile([128, NTB, 8, TB], f32r, tag="xq", name="xq")
            v_sb = resid.tile([128, nkt, HL, D + 1], f32r)
            # ones columns for the denominator rows: fill the whole tile,
            # the v-projection copies then overwrite the [., ., ., 0:D] part
            if bf16:
                nc.vector.memset(v_sb, 1.0)
            else:
                nc.vector.memset(v_sb[:, :, :, :].bitcast(f32), 1.0)

            # ---- k projection (per key-block, so attention can begin
            # after kb0) ----
            def emit_xk(kb_i):
                kbw = kbs[kb_i]
                xk_t = xs.tile([128, 6, 512], f32r, tag="xk", name="xk_t", bufs=3)
                nc.sync.dma_start(
                    out=xk_t[:, :, :kbw], in_=XK[:, kb_i, :, :kbw]
                )
                return xk_t

            def emit_kproj_m(kb_i, m, xk_t):
                kbw = kbs[kb_i]
                s0 = kb_i * 512
                ps_t = ps.tile([128, 512], f32, tag="proj", name="kp_ps")
                for k in range(6):
                    nc.tensor.matmul(
                        ps_t[:, :kbw],
                        wk_sb[:, k, m * 128 : (m + 1) * 128],
                        xk_t[:, k, :kbw],
                        start=(k == 0),
                        stop=(k == 5),
                    )
                nc.vector.tensor_scalar_add(
                    kT[m][:, s0 : s0 + kbw],
                    ps_t[:, :kbw],
                    bqk_sb[:, 2 + m : 3 + m],
                )

            # ---- v projection (emitted later for tb0/p0 split; see
            # emit_vproj) ----
            vproj_state = {}

            def emit_xv(kb_i):
                if kb_i not in vproj_state:
                    kbw = kbs[kb_i]
                    xv_t = xs.tile(
                        [128, 6, 512], f32r, tag="xv", name="xv_t",
                        bufs=max(2, len(kbs)),
                    )
                    nc.sync.dma_start(
                        out=xv_t[:, :, :kbw], in_=XV[:, kb_i, :, :kbw]
                    )
                    vproj_state[kb_i] = xv_t
                return vproj_state[kb_i]

            def emit_vproj_kt(kt):
                kb_i = kt // 4
                sub = kt % 4
                xv_t = emit_xv(kb_i)
                ps_t = ps.tile([128, 512], f32, tag="proj", name="vp_ps")
                for k in range(6):
                    nc.tensor.matmul(
                        ps_t[:, :GH],
                        xv_t[:, k, sub * 128 : (sub + 1) * 128],
                        wv_sb[:, k, :],
                        start=(k == 0),
                        stop=(k == 5),
                    )
                nc.vector.tensor_copy(
                    v_sb[:, kt, :, 0:D],
                    ps_t[:, :GH].rearrange("p (h d) -> p h d", h=HL),
                )

            def emit_xq(tb, half=None):
                if half is None:
                    nc.sync.dma_start(out=xq_all[:, tb], in_=XQ[:, tb])
                else:
                    k0, k1 = (0, 4) if half == 0 else (4, 8)
                    nc.sync.dma_start(
                        out=xq_all[:, tb, k0:k1], in_=XQ[:, tb, k0:k1]
                    )

            def emit_qproj_m(tb, m):
                t0 = tb * TB
                ps_t = ps.tile([128, 512], f32, tag="proj", name="qp_ps")
                for k in range(8):
                    nc.tensor.matmul(
                        ps_t,
                        wq_sb[:, k, m * 128 : (m + 1) * 128],
                        xq_all[:, tb, k, :],
                        start=(k == 0),
                        stop=(k == 7),
                    )
                nc.vector.tensor_scalar_add(
                    qT[m][:, t0 : t0 + TB],
                    ps_t,
                    bqk_sb[:, m : m + 1],
                )

            def emit_scores(p, tb, kt, prtag, prbufs):
                t0 = tb * TB
                k0 = kt * 128
                sc = ps.tile([128, 2, TB], f32, tag="sc", name="sc")
                for hh in range(2):
                    nc.tensor.matmul(
                        sc[:, hh, :],
                        kT[p][hh * 64 : hh * 64 + 64, k0 : k0 + 128],
                        qT[p][hh * 64 : hh * 64 + 64, t0 : t0 + TB],
                        start=True,
                        stop=True,
                        tile_position=(hh * 64, 0),
                    )
                pr = probs_pool.tile(
                    [128, 2, TB], f32r, tag=prtag, name="pr", bufs=prbufs
                )
                nc.scalar.activation(
                    pr, sc, Exp, bias=mb_sb[:, kt : kt + 1], scale=0.125
                )
                return pr

            def emit_pv(p, tb, kt, pr, ctx_ps):
                for hh in range(2):
                    nc.tensor.matmul(
                        ctx_ps[hh],
                        v_sb[:, kt, 2 * p + hh, :],
                        pr[:, hh, :],
                        start=(kt == 0),
                        stop=(kt == nkt - 1),
                    )

            def emit_normalize(p, tb, ctx_ps):
                t0 = tb * TB
                # evacuate both PSUM ctx tiles to SBUF first (quick DVE
                # copies) so the next block's PV matmuls get their PSUM
                # slots immediately; the normalize chain then runs from
                # SBUF off the PE critical path
                evac = []
                for hh in range(2):
                    ctmp = norm_pool.tile([D, TB], f32, tag="ctmp", name="ctmp")
                    nc.vector.tensor_copy(ctmp, ctx_ps[hh][0:D, :])
                    dcp = norm_pool.tile([1, TB], f32, tag="dcp", name="dcp")
                    nc.vector.tensor_copy(dcp, ctx_ps[hh][D : D + 1, :])
                    evac.append((ctmp, dcp))
                for hh in range(2):
                    ctmp, dcp = evac[hh]
                    rbc = norm_pool.tile([D, TB], f32, tag="rbc", name="rbc")
                    nc.gpsimd.partition_broadcast(rbc, dcp)
                    rec = norm_pool.tile([D, TB], f32, tag="rec", name="rec")
                    nc.vector.reciprocal_approx_fast(out=rec, in_=rbc)
                    dst = qT[p][hh * 64 : hh * 64 + 64, t0 : t0 + TB]
                    nc.vector.tensor_mul(dst, ctmp, rec)
                    if with_bv:
                        nc.vector.tensor_scalar_add(
                            dst, dst, bv_sb[64 * hh : 64 * hh + 64, p : p + 1]
                        )

            def emit_attn(p, tb, thunks=(), prs=None):
                # kt processed in pairs: adjacent score pairs keep their
                # LDWEIGHTS together (interleaved full-height thunk loads
                # otherwise split the row-tiled pair concurrency) and the
                # exp of each kt gets a full pair-cycle of lead time
                # before its PV consumes it
                thunks = list(thunks)
                ti = 0

                def run_thunk():
                    nonlocal ti
                    if ti < len(thunks):
                        thunks[ti]()
                        ti += 1

                ctx_ps = [
                    ps.tile([D + 1, TB], f32, tag="ctx", name=f"ctx{p}_{tb}_{i}")
                    for i in range(2)
                ]
                for kt0 in range(0, nkt, 2):
                    kts = [kt0] + ([kt0 + 1] if kt0 + 1 < nkt else [])
                    prl = []
                    for kt in kts:
                        prl.append(
                            prs[kt] if prs is not None
                            else emit_scores(p, tb, kt, "pr", 4)
                        )
                    run_thunk()
                    for i, kt in enumerate(kts):
                        emit_pv(p, tb, kt, prl[i], ctx_ps)
                    run_thunk()
                while ti < len(thunks):
                    thunks[ti]()
                    ti += 1
                emit_normalize(p, tb, ctx_ps)

            def qproj_thunks(tb):
                # 6 small thunks: 3+3+2 matmuls per m-half, chained into
                # one psum accumulation (interleaving with other banks is
                # legal), evac on the last
                cell = {}

                def chunk(m, k0, k1, fin, tb=tb):
                    if k0 == 0:
                        cell[m] = ps.tile(
                            [128, 512], f32, tag="proj", name="qp_ps"
                        )
                    ps_t = cell[m]
                    for k in range(k0, k1):
                        nc.tensor.matmul(
                            ps_t,
                            wq_sb[:, k, m * 128 : (m + 1) * 128],
                            xq_all[:, tb, k, :],
                            start=(k == 0),
                            stop=(k == 7),
                        )
                    if fin:
                        nc.vector.tensor_scalar_add(
                            qT[m][:, tb * TB : (tb + 1) * TB],
                            ps_t,
                            bqk_sb[:, m : m + 1],
                        )

                out = []
                for m in range(2):
                    out += [
                        lambda m=m: chunk(m, 0, 3, False),
                        lambda m=m: chunk(m, 3, 6, False),
                        lambda m=m: chunk(m, 6, 8, True),
                    ]
                return out

            def outproj_pieces(tb):
                out = []
                for tt in range(4 * tb, 4 * tb + 4):
                    for nh in range(2):
                        out.append(
                            lambda tt=tt, nh=nh: emit_outproj_nh(tt, nh)
                        )
                return out

            def emit_outproj_nh(tt, nh):
                # evac on DVE (keeps ACT free for the exps) and DMA on the
                # vector HWDGE ring so the sync ring stays dedicated to the
                # big input streams
                ps_t = ps.tile([128, 512], f32, tag="proj", name="op_ps")
                for kk in range(2):
                    nc.tensor.matmul(
                        ps_t,
                        qT[kk][:, tt * 128 : (tt + 1) * 128],
                        wo_sb[:, kk, nh * 512 : (nh + 1) * 512],
                        start=(kk == 0),
                        stop=(kk == 1),
                    )
                o_sb = outs_pool.tile(
                    [128, 512], f32r, tag="osb", name="o_sb"
                )
                nc.vector.tensor_copy(o_sb, ps_t)
                nc.sync.dma_start(out=OUT[tt, nh], in_=o_sb)

            def emit_outproj(tb):
                for tt in range(4 * tb, 4 * tb + 4):
                    for nh in range(2):
                        emit_outproj_nh(tt, nh)

            # ---- emission schedule ----
            # DMA priority order on the sync ring: xk0, xq0, xk1.., xv
            # blocks, then the remaining xq tiles — everything is issued
            # up front so out-DMAs appended later never delay inputs.
            # Preamble: minimal PE path to the first exp (kb0/m0 + q/m0 +
            # scores kt0..3); steady state: every attention group carries
            # PE filler thunks (qproj of tb+1 or outproj of tb-1) so the
            # PE never runs a matmul-only phase while ACT idles
            xk_ts = [emit_xk(0)]
            nc.sync.dma_start(out=wq_sb, in_=WQ)
            emit_xq(0, half=0)
            emit_xq(0, half=1)
            for kb_i in range(1, len(kbs)):
                xk_ts.append(emit_xk(kb_i))
            for kb_i in range(len(kbs)):
                emit_xv(kb_i)
            for tb in range(1, NTB):
                emit_xq(tb)
            emit_kproj_m(0, 0, xk_ts[0])
            emit_qproj_m(0, 0)
            kt_kb0 = min(4, nkt)
            prs0 = [emit_scores(0, 0, kt, "pr0", nkt) for kt in range(kt_kb0)]
            emit_kproj_m(0, 1, xk_ts[0])
            emit_qproj_m(0, 1)
            for kb_i in range(1, len(kbs)):
                emit_kproj_m(kb_i, 0, xk_ts[kb_i])
                emit_kproj_m(kb_i, 1, xk_ts[kb_i])
            prs0 += [
                emit_scores(0, 0, kt, "pr0", nkt) for kt in range(kt_kb0, nkt)
            ]
            for kt in range(nkt):
                emit_vproj_kt(kt)
            # p=1 scores are emitted as thunks inside the p=0 PV pass so
            # their exps keep ACT busy through the matmul-only stretch
            prs1 = []

            def scores1_thunks():
                return [
                    (lambda kt=kt: prs1.append(
                        emit_scores(1, 0, kt, "pr1", nkt)
                    ))
                    for kt in range(nkt)
                ]

            emit_attn(0, 0, prs=prs0, thunks=scores1_thunks())
            emit_attn(1, 0, thunks=qproj_thunks(1), prs=prs1)
            # thunk placement: qproj pieces (no recent deps) take the early
            # slots; outproj(tb-1) pieces wait on normalize(1, tb-1) which
            # only completes ~2us into attn(0, tb), so they go late in
            # attn(0, tb) and into attn(1, tb)
            noop = lambda: None  # noqa: E731
            for tb in range(1, NTB):
                op = outproj_pieces(tb - 1)
                if tb < NTB - 1:
                    emit_attn(0, tb, thunks=qproj_thunks(tb + 1) + op[:2])
                    emit_attn(1, tb, thunks=op[2:])
                else:
                    emit_attn(0, tb, thunks=[noop, noop] + op[:6])
                    emit_attn(1, tb, thunks=op[6:])
            emit_outproj(NTB - 1)

    nc.compile()
    return nc


def kernel(
    query, key, value, Wq, bq, Wk, bk, Wv, bv, Wo, bo, query_mask, key_mask
):
    global LAST_EXEC_NS, LAST_TRACE_DIR
    from concourse.bass_utils import run_bass_kernel_spmd

    query = np.asarray(query, dtype=np.float32)
    key = np.asarray(key, dtype=np.float32)
    value = np.asarray(value, dtype=np.float32)
    Wq = np.asarray(Wq, dtype=np.float32)
    Wk = np.asarray(Wk, dtype=np.float32)
    Wv = np.asarray(Wv, dtype=np.float32)
    Wo = np.asarray(Wo, dtype=np.float32)
    bq = np.asarray(bq, dtype=np.float32)
    bk = np.asarray(bk, dtype=np.float32)
    bv = np.asarray(bv, dtype=np.float32)
    bo = np.asarray(bo, dtype=np.float32)
    qm = np.asarray(query_mask)
    km = np.asarray(key_mask)

    # host-side key compaction (query_mask masks the KEY axis, globally
    # per batch)
    keep = [np.flatnonzero(qm[b] != 0) for b in range(B)]
    nkeep = max((len(k) for k in keep), default=0)
    nkt = max(1, math.ceil(nkeep / 128))
    nkeys = nkt * 128

    with_bv = bool(np.any(bv))
    ck = (nkt, with_bv, BF16)
    if ck not in _CACHE:
        _CACHE[ck] = _build(nkt, with_bv, BF16)
    nc = _CACHE[ck]

    wdt = ml_dtypes.bfloat16 if BF16 else np.float32
    nkb = math.ceil(nkeys / 512)
    nkeys_b = nkb * 512

    def arr_kmajor(a, ktiles):  # [dim, n] -> [128, ktiles, n]
        return np.ascontiguousarray(
            a.reshape(ktiles, 128, a.shape[1]).transpose(1, 0, 2)
        ).astype(wdt)

    def arr_blocked(a, ktiles, blocks):  # [dim, n] -> [128, blocks, kt, 512]
        return np.ascontiguousarray(
            a.reshape(ktiles, 128, blocks, 512).transpose(1, 2, 0, 3)
        ).astype(wdt)

    in_maps = []
    for c in range(N_CORES):
        b, hg = c // HG, c % HG
        hs = hg * GH
        idx = keep[b]
        # compacted + padded key/value (transposed)
        xk = np.zeros((KDIM, nkeys_b), np.float32)
        xk[:, : len(idx)] = key[b].T[:, idx]
        xv = np.zeros((VDIM, nkeys_b), np.float32)
        xv[:, : len(idx)] = value[b].T[:, idx]
        mbias = np.full((nkeys,), NEG, np.float32)
        mbias[: len(idx)] = 0.0
        bqk = np.empty((128, 4), np.float32)
        bqk[:, 0] = bq[hs : hs + 128]
        bqk[:, 1] = bq[hs + 128 : hs + 256]
        bqk[:, 2] = bk[hs : hs + 128]
        bqk[:, 3] = bk[hs + 128 : hs + 256]
        m = {
            "xq": arr_blocked(query[b].T, 8, NTB),
            "xk": arr_blocked(xk, 6, nkb),
            "xv": arr_blocked(xv, 6, nkb),
            "wq": arr_kmajor(Wq[:, hs : hs + GH], 8),
            "wk": arr_kmajor(Wk[:, hs : hs + GH], 6),
            "wv": arr_kmajor(Wv[:, hs : hs + GH], 6),
            "wo": arr_kmajor(Wo[hs : hs + GH, :], 2),
            "mbias": np.ascontiguousarray(mbias.reshape(nkt, 128).T),
            "bqk": bqk,
        }
        if with_bv:
            bvt = np.empty((128, 2), np.float32)
            bvt[:, 0] = bv[hs : hs + 128]
            bvt[:, 1] = bv[hs + 128 : hs + 256]
            m["bv"] = bvt
        in_maps.append(m)

    kwargs = {}
    if PROFILE:
        import tempfile

        LAST_TRACE_DIR = tempfile.mkdtemp(prefix="bass_trace_")
        kwargs = {"trace": True, "tmpdir": LAST_TRACE_DIR}
    res = run_bass_kernel_spmd(nc, in_maps, list(range(N_CORES)), **kwargs)
    LAST_EXEC_NS = res.exec_time_ns

    out = np.zeros((B, LQ, QDIM), np.float32)
    for c in range(N_CORES):
        blk = res.results[c]["outp"]  # [LQ//128, 2, 128, 512] block-major
        out[c // HG] += blk.transpose(0, 2, 1, 3).reshape(LQ, QDIM)
    out += bo[None, None, :]
    for b in range(B):
        if len(keep[b]) == 0:
            # all keys masked: reference softmax is NaN everywhere
            out[b] = np.nan
    # key_mask masks the QUERY axis in the reference; a zero row makes the
    # whole softmax row -inf -> NaN output for that query position.
    for b in range(B):
        zq = np.flatnonzero(km[b] == 0)
        if len(zq):
            out[b, zq, :] = np.nan
    return out



# revision 38
# speedup vs baseline: 1.1625x; 1.0221x over previous
"""Trainium2 Bass kernel for nn_CrossModalAttention (B=2, LQ=LK=2048,
QDIM=HID=1024, KDIM=VDIM=768, H=16, D=64).

Sharding: 8 cores = 2 batches x 4 head-groups (4 heads each).
Per core: q/k/v projections column-sliced over HID, attention for its 4
heads, row-parallel partial of the out-projection. Host sums the 4
partials per batch (the row-parallel unshard) and adds bo.

Device dataflow (per core), all matmuls in fp32r (TF32-like, ~1.5e-4):
  - host passes query/key/value[b] transposed (and K/V key-compacted:
    query_mask masks the KEY axis globally per batch, so masked keys are
    dropped on host and the remainder padded to a multiple of 128)
  - qT/kT [hid, tokens] and v [keys, hid] computed on device
  - per head pair (row-packed K=64 matmuls via tile_position):
    scoresT [keys, q] -> ACT exp(s/8 + mask_bias) -> PV matmul with a
    ones-augmented V (M=65) giving ctxT and the softmax denominator
  - normalize on DVE (reciprocal + gpsimd partition-broadcast)
  - out-projection from ctxT, partial written to DRAM
"""

import math

import ml_dtypes
import numpy as np

B, LQ, LK = 2, 2048, 2048
QDIM, KDIM, VDIM, HID, H = 1024, 768, 768, 1024, 16
D = HID // H  # 64
HG = 4  # head-groups (cores per batch)
HL = H // HG  # heads per core = 4
GH = HL * D  # per-core hid slice = 256
N_CORES = 8
TB = 512  # token block
NTB = LQ // TB  # 4
NEG = -1.0e30

BF16 = True
PROFILE = False
LAST_EXEC_NS = None
LAST_TRACE_DIR = None

_CACHE = {}


def _build(nkt: int, with_bv: bool, bf16: bool):
    import concourse.bacc as bacc
    import concourse.mybir as mybir
    import concourse.tile as tile

    nkeys = nkt * 128
    # key blocks of <=512 for the k-projection
    kbs = [min(512, nkeys - s) for s in range(0, nkeys, 512)]

    f32 = mybir.dt.float32
    f32r = mybir.dt.bfloat16 if bf16 else mybir.dt.float32r
    Exp = mybir.ActivationFunctionType.Exp
    Ident = mybir.ActivationFunctionType.Identity

    nc = bacc.Bacc(
        "TRN2", target_bir_lowering=False, debug=False, num_devices=N_CORES
    )

    nkb = len(kbs)
    # DRAM tensors (per-core shapes). Inputs are laid out block-major so
    # every DMA moves large contiguous per-partition lines (1KB strided
    # lines are descriptor-rate-bound: ~15us for 1MB)
    XQ = nc.dram_tensor(
        "xq", [128, NTB, 8, TB], f32r, kind="ExternalInput"
    ).ap()
    XK = nc.dram_tensor(
        "xk", [128, nkb, 6, 512], f32r, kind="ExternalInput"
    ).ap()
    XV = nc.dram_tensor(
        "xv", [128, nkb, 6, 512], f32r, kind="ExternalInput"
    ).ap()
    WQ = nc.dram_tensor("wq", [128, 8, GH], f32r, kind="ExternalInput").ap()
    WK = nc.dram_tensor("wk", [128, 6, GH], f32r, kind="ExternalInput").ap()
    WV = nc.dram_tensor("wv", [128, 6, GH], f32r, kind="ExternalInput").ap()
    WO = nc.dram_tensor("wo", [128, 2, QDIM], f32r, kind="ExternalInput").ap()
    MB = nc.dram_tensor("mbias", [128, nkt], f32, kind="ExternalInput").ap()
    BQ = nc.dram_tensor("bqk", [128, 4], f32, kind="ExternalInput").ap()
    BV = None
    if with_bv:
        BV = nc.dram_tensor("bv", [128, 2], f32, kind="ExternalInput").ap()
    # output block-major: [tt, nh, 128, 512] so each store is one fully
    # contiguous write; bf16 halves the drain (host sums partials in f32)
    OUT = nc.dram_tensor(
        "outp", [LQ // 128, 2, 128, 512], f32r, kind="ExternalOutput"
    ).ap()

    with tile.TileContext(nc) as tc:
        with (
            tc.tile_pool(name="consts", bufs=1) as consts,
            tc.tile_pool(name="resid", bufs=1) as resid,
            tc.tile_pool(name="xs", bufs=2) as xs,
            tc.tile_pool(name="probs", bufs=4) as probs_pool,
            tc.tile_pool(name="norm", bufs=3) as norm_pool,
            tc.tile_pool(name="outs", bufs=3) as outs_pool,
            tc.tile_pool(name="ps", bufs=2, space="PSUM") as ps,
        ):
            # ---- constants / weights ----
            # wk/wq gate the very first matmuls: they go FIRST on the sync
            # HWDGE ring (the gpsimd SWDGE ring takes ~7us to complete a
            # transfer, which used to stall the PE until ~15us). wv/wo are
            # needed later and stay on the SWDGE ring.
            wq_sb = consts.tile([128, 8, GH], f32r)
            wk_sb = consts.tile([128, 6, GH], f32r)
            wv_sb = consts.tile([128, 6, GH], f32r)
            wo_sb = consts.tile([128, 2, QDIM], f32r)
            mb_sb = consts.tile([128, nkt], f32)
            bqk_sb = consts.tile([128, 4], f32)
            nc.sync.dma_start(out=wk_sb, in_=WK)
            nc.scalar.dma_start(out=bqk_sb, in_=BQ)
            nc.scalar.dma_start(out=mb_sb, in_=MB)
            nc.gpsimd.dma_start(out=wv_sb, in_=WV)
            nc.gpsimd.dma_start(out=wo_sb, in_=WO)
            bv_sb = None
            if with_bv:
                bv_sb = consts.tile([128, 2], f32)
                nc.gpsimd.dma_start(out=bv_sb, in_=BV)

            # ---- residents ----
            # qT tiles double as ctxT tiles later (WAR handled by Tile)
            qT = [resid.tile([128, LQ], f32r, tag=f"qT{p}", name=f"qT{p}") for p in range(2)]
            kT = [resid.tile([128, nkeys], f32r, tag=f"kT{p}", name=f"kT{p}") for p in range(2)]
            xq_all = resid.tile([128, NTB, 8, TB], f32r, tag="xq", name="xq")
            v_sb = resid.tile([128, nkt, HL, D + 1], f32r)
            # ones columns for the denominator rows: fill the whole tile,
            # the v-projection copies then overwrite the [., ., ., 0:D] part
            if bf16:
                nc.vector.memset(v_sb, 1.0)
            else:
                nc.vector.memset(v_sb[:, :, :, :].bitcast(f32), 1.0)

            # ---- k projection (per key-block, so attention can begin
            # after kb0) ----
            def emit_xk(kb_i):
                kbw = kbs[kb_i]
                xk_t = xs.tile([128, 6, 512], f32r, tag="xk", name="xk_t", bufs=3)
                nc.sync.dma_start(
                    out=xk_t[:, :, :kbw], in_=XK[:, kb_i, :, :kbw]
                )
                return xk_t

            def emit_kproj_m(kb_i, m, xk_t):
                kbw = kbs[kb_i]
                s0 = kb_i * 512
                ps_t = ps.tile([128, 512], f32, tag="proj", name="kp_ps")
                for k in range(6):
                    nc.tensor.matmul(
                        ps_t[:, :kbw],
                        wk_sb[:, k, m * 128 : (m + 1) * 128],
                        xk_t[:, k, :kbw],
                        start=(k == 0),
                        stop=(k == 5),
                    )
                nc.vector.tensor_scalar_add(
                    kT[m][:, s0 : s0 + kbw],
                    ps_t[:, :kbw],
                    bqk_sb[:, 2 + m : 3 + m],
                )

            # ---- v projection (emitted later for tb0/p0 split; see
            # emit_vproj) ----
            vproj_state = {}

            def emit_xv(kb_i):
                if kb_i not in vproj_state:
                    kbw = kbs[kb_i]
                    xv_t = xs.tile(
                        [128, 6, 512], f32r, tag="xv", name="xv_t",
                        bufs=max(2, len(kbs)),
                    )
                    nc.sync.dma_start(
                        out=xv_t[:, :, :kbw], in_=XV[:, kb_i, :, :kbw]
                    )
                    vproj_state[kb_i] = xv_t
                return vproj_state[kb_i]

            def emit_vproj_kt(kt):
                kb_i = kt // 4
                sub = kt % 4
                xv_t = emit_xv(kb_i)
                ps_t = ps.tile([128, 512], f32, tag="proj", name="vp_ps")
                for k in range(6):
                    nc.tensor.matmul(
                        ps_t[:, :GH],
                        xv_t[:, k, sub * 128 : (sub + 1) * 128],
                        wv_sb[:, k, :],
                        start=(k == 0),
                        stop=(k == 5),
                    )
                nc.vector.tensor_copy(
                    v_sb[:, kt, :, 0:D],
                    ps_t[:, :GH].rearrange("p (h d) -> p h d", h=HL),
                )

            def emit_xq(tb, half=None):
                if half is None:
                    nc.sync.dma_start(out=xq_all[:, tb], in_=XQ[:, tb])
                else:
                    k0, k1 = (0, 4) if half == 0 else (4, 8)
                    nc.sync.dma_start(
                        out=xq_all[:, tb, k0:k1], in_=XQ[:, tb, k0:k1]
                    )

            def emit_qproj_m(tb, m):
                t0 = tb * TB
                ps_t = ps.tile([128, 512], f32, tag="proj", name="qp_ps")
                for k in range(8):
                    nc.tensor.matmul(
                        ps_t,
                        wq_sb[:, k, m * 128 : (m + 1) * 128],
                        xq_all[:, tb, k, :],
                        start=(k == 0),
                        stop=(k == 7),
                    )
                nc.vector.tensor_scalar_add(
                    qT[m][:, t0 : t0 + TB],
                    ps_t,
                    bqk_sb[:, m : m + 1],
                )

            def emit_scores(p, tb, kt, prtag, prbufs):
                t0 = tb * TB
                k0 = kt * 128
                sc = ps.tile([128, 2, TB], f32, tag="sc", name="sc")
                for hh in range(2):
                    nc.tensor.matmul(
                        sc[:, hh, :],
                        kT[p][hh * 64 : hh * 64 + 64, k0 : k0 + 128],
                        qT[p][hh * 64 : hh * 64 + 64, t0 : t0 + TB],
                        start=True,
                        stop=True,
                        tile_position=(hh * 64, 0),
                    )
                pr = probs_pool.tile(
                    [128, 2, TB], f32r, tag=prtag, name="pr", bufs=prbufs
                )
                nc.scalar.activation(
                    pr, sc, Exp, bias=mb_sb[:, kt : kt + 1], scale=0.125
                )
                return pr

            def emit_pv_hh(p, tb, kt, pr, ctx_ps, hh):
                nc.tensor.matmul(
                    ctx_ps[hh],
                    v_sb[:, kt, 2 * p + hh, :],
                    pr[:, hh, :],
                    start=(kt == 0),
                    stop=(kt == nkt - 1),
                )

            def emit_normalize(p, tb, ctx_ps):
                t0 = tb * TB
                # evacuate both PSUM ctx tiles to SBUF first (quick DVE
                # copies) so the next block's PV matmuls get their PSUM
                # slots immediately; the normalize chain then runs from
                # SBUF off the PE critical path
                evac = []
                for hh in range(2):
                    ctmp = norm_pool.tile([D, TB], f32, tag="ctmp", name="ctmp")
                    nc.vector.tensor_copy(ctmp, ctx_ps[hh][0:D, :])
                    dcp = norm_pool.tile([1, TB], f32, tag="dcp", name="dcp")
                    nc.vector.tensor_copy(dcp, ctx_ps[hh][D : D + 1, :])
                    evac.append((ctmp, dcp))
                for hh in range(2):
                    ctmp, dcp = evac[hh]
                    rbc = norm_pool.tile([D, TB], f32, tag="rbc", name="rbc")
                    nc.gpsimd.partition_broadcast(rbc, dcp)
                    rec = norm_pool.tile([D, TB], f32, tag="rec", name="rec")
                    nc.vector.reciprocal_approx_fast(out=rec, in_=rbc)
                    dst = qT[p][hh * 64 : hh * 64 + 64, t0 : t0 + TB]
                    nc.vector.tensor_mul(dst, ctmp, rec)
                    if with_bv:
                        nc.vector.tensor_scalar_add(
                            dst, dst, bv_sb[64 * hh : 64 * hh + 64, p : p + 1]
                        )

            def emit_attn2(p, tb, prs, thunks=()):
                # software-pipelined group: the probs for (p, tb) were
                # fully computed during the PREVIOUS group, so the PV
                # matmuls here never wait on an exp. The thunks (next
                # group's scores + a projection slice) provide the PE
                # stream and ACT feed for the NEXT group.
                thunks = list(thunks)
                ti = 0

                def run_thunk():
                    nonlocal ti
                    if ti < len(thunks):
                        thunks[ti]()
                        ti += 1

                ctx_ps = [
                    ps.tile([D + 1, TB], f32, tag="ctx", name=f"ctx{p}_{tb}_{i}")
                    for i in range(2)
                ]
                for kt in range(nkt):
                    run_thunk()
                    emit_pv_hh(p, tb, kt, prs[kt], ctx_ps, 0)
                    run_thunk()
                    emit_pv_hh(p, tb, kt, prs[kt], ctx_ps, 1)
                while ti < len(thunks):
                    thunks[ti]()
                    ti += 1
                emit_normalize(p, tb, ctx_ps)

            def qproj_thunks(tb):
                # 6 small thunks: 3+3+2 matmuls per m-half, chained into
                # one psum accumulation (interleaving with other banks is
                # legal), evac on the last
                cell = {}

                def chunk(m, k0, k1, fin, tb=tb):
                    if k0 == 0:
                        cell[m] = ps.tile(
                            [128, 512], f32, tag="proj", name="qp_ps"
                        )
                    ps_t = cell[m]
                    for k in range(k0, k1):
                        nc.tensor.matmul(
                            ps_t,
                            wq_sb[:, k, m * 128 : (m + 1) * 128],
                            xq_all[:, tb, k, :],
                            start=(k == 0),
                            stop=(k == 7),
                        )
                    if fin:
                        nc.vector.tensor_scalar_add(
                            qT[m][:, tb * TB : (tb + 1) * TB],
                            ps_t,
                            bqk_sb[:, m : m + 1],
                        )

                out = []
                for m in range(2):
                    out += [
                        lambda m=m: chunk(m, 0, 3, False),
                        lambda m=m: chunk(m, 3, 6, False),
                        lambda m=m: chunk(m, 6, 8, True),
                    ]
                return out

            def outproj_pieces(tb):
                out = []
                for tt in range(4 * tb, 4 * tb + 4):
                    for nh in range(2):
                        out.append(
                            lambda tt=tt, nh=nh: emit_outproj_nh(tt, nh)
                        )
                return out

            def emit_outproj_nh(tt, nh):
                # evac on DVE (keeps ACT free for the exps) and DMA on the
                # vector HWDGE ring so the sync ring stays dedicated to the
                # big input streams
                ps_t = ps.tile([128, 512], f32, tag="proj", name="op_ps")
                for kk in range(2):
                    nc.tensor.matmul(
                        ps_t,
                        qT[kk][:, tt * 128 : (tt + 1) * 128],
                        wo_sb[:, kk, nh * 512 : (nh + 1) * 512],
                        start=(kk == 0),
                        stop=(kk == 1),
                    )
                o_sb = outs_pool.tile(
                    [128, 512], f32r, tag="osb", name="o_sb"
                )
                nc.vector.tensor_copy(o_sb, ps_t)
                nc.sync.dma_start(out=OUT[tt, nh], in_=o_sb)

            def emit_outproj(tb):
                for tt in range(4 * tb, 4 * tb + 4):
                    for nh in range(2):
                        emit_outproj_nh(tt, nh)

            # ---- emission schedule ----
            # DMA priority order on the sync ring: xk0, xq0, xk1.., xv
            # blocks, then the remaining xq tiles — everything is issued
            # up front so out-DMAs appended later never delay inputs.
            # Preamble: minimal PE path to the first exp (kb0/m0 + q/m0 +
            # scores kt0..3); steady state: every attention group carries
            # PE filler thunks (qproj of tb+1 or outproj of tb-1) so the
            # PE never runs a matmul-only phase while ACT idles
            xk_ts = [emit_xk(0)]
            nc.sync.dma_start(out=wq_sb, in_=WQ)
            emit_xq(0, half=0)
            emit_xq(0, half=1)
            for kb_i in range(1, len(kbs)):
                xk_ts.append(emit_xk(kb_i))
            for kb_i in range(len(kbs)):
                emit_xv(kb_i)
            for tb in range(1, NTB):
                emit_xq(tb)
            emit_kproj_m(0, 0, xk_ts[0])
            emit_qproj_m(0, 0)
            kt_kb0 = min(4, nkt)
            prs0 = [emit_scores(0, 0, kt, "pr0", nkt) for kt in range(kt_kb0)]
            emit_kproj_m(0, 1, xk_ts[0])
            emit_qproj_m(0, 1)
            for kb_i in range(1, len(kbs)):
                emit_kproj_m(kb_i, 0, xk_ts[kb_i])
                emit_kproj_m(kb_i, 1, xk_ts[kb_i])
            prs0 += [
                emit_scores(0, 0, kt, "pr0", nkt) for kt in range(kt_kb0, nkt)
            ]
            for kt in range(nkt):
                emit_vproj_kt(kt)

            # one-group software pipeline: group gi's PV pass carries the
            # scores of group gi+1 (alternating pr0/pr1 generations) plus
            # a projection slice as thunks. qproj(tb+1) rides the (0, tb)
            # group; outproj(tb-1) rides (1, tb) — a full group after the
            # normalize it consumes, so it never stalls the PE queue.
            order = [(p_, tb_) for tb_ in range(NTB) for p_ in (0, 1)]
            prs_map = {0: prs0}

            def mk_scores_thunks(gi):
                p_, tb_ = order[gi]
                store = prs_map.setdefault(gi, [])
                tag = "pr0" if gi % 2 == 0 else "pr1"
                return [
                    (lambda kt=kt: store.append(
                        emit_scores(p_, tb_, kt, tag, nkt)
                    ))
                    for kt in range(nkt)
                ]

            def interleave(a, b):
                out = []
                for i in range(max(len(a), len(b))):
                    if i < len(a):
                        out.append(a[i])
                    if i < len(b):
                        out.append(b[i])
                return out

            for gi, (p_, tb_) in enumerate(order):
                sth = mk_scores_thunks(gi + 1) if gi + 1 < len(order) else []
                if p_ == 0:
                    proj = (
                        qproj_thunks(tb_ + 1) if tb_ + 1 < NTB else []
                    )
                else:
                    proj = outproj_pieces(tb_ - 1) if tb_ >= 1 else []
                emit_attn2(p_, tb_, prs_map[gi], interleave(sth, proj))
            emit_outproj(NTB - 1)

    nc.compile()
    return nc


def kernel(
    query, key, value, Wq, bq, Wk, bk, Wv, bv, Wo, bo, query_mask, key_mask
):
    global LAST_EXEC_NS, LAST_TRACE_DIR
    from concourse.bass_utils import run_bass_kernel_spmd

    query = np.asarray(query, dtype=np.float32)
    key = np.asarray(key, dtype=np.float32)
    value = np.asarray(value, dtype=np.float32)
    Wq = np.asarray(Wq, dtype=np.float32)
    Wk = np.asarray(Wk, dtype=np.float32)
    Wv = np.asarray(Wv, dtype=np.float32)
    Wo = np.asarray(Wo, dtype=np.float32)
    bq = np.asarray(bq, dtype=np.float32)
    bk = np.asarray(bk, dtype=np.float32)
    bv = np.asarray(bv, dtype=np.float32)
    bo = np.asarray(bo, dtype=np.float32)
    qm = np.asarray(query_mask)
    km = np.asarray(key_mask)

    # host-side key compaction (query_mask masks the KEY axis, globally
    # per batch)
    keep = [np.flatnonzero(qm[b] != 0) for b in range(B)]
    nkeep = max((len(k) for k in keep), default=0)
    nkt = max(1, math.ceil(nkeep / 128))
    nkeys = nkt * 128

    with_bv = bool(np.any(bv))
    ck = (nkt, with_bv, BF16)
    if ck not in _CACHE:
        _CACHE[ck] = _build(nkt, with_bv, BF16)
    nc = _CACHE[ck]

    wdt = ml_dtypes.bfloat16 if BF16 else np.float32
    nkb = math.ceil(nkeys / 512)
    nkeys_b = nkb * 512

    def arr_kmajor(a, ktiles):  # [dim, n] -> [128, ktiles, n]
        return np.ascontiguousarray(
            a.reshape(ktiles, 128, a.shape[1]).transpose(1, 0, 2)
        ).astype(wdt)

    def arr_blocked(a, ktiles, blocks):  # [dim, n] -> [128, blocks, kt, 512]
        return np.ascontiguousarray(
            a.reshape(ktiles, 128, blocks, 512).transpose(1, 2, 0, 3)
        ).astype(wdt)

    in_maps = []
    for c in range(N_CORES):
        b, hg = c // HG, c % HG
        hs = hg * GH
        idx = keep[b]
        # compacted + padded key/value (transposed)
        xk = np.zeros((KDIM, nkeys_b), np.float32)
        xk[:, : len(idx)] = key[b].T[:, idx]
        xv = np.zeros((VDIM, nkeys_b), np.float32)
        xv[:, : len(idx)] = value[b].T[:, idx]
        mbias = np.full((nkeys,), NEG, np.float32)
        mbias[: len(idx)] = 0.0
        bqk = np.empty((128, 4), np.float32)
        bqk[:, 0] = bq[hs : hs + 128]
        bqk[:, 1] = bq[hs + 128 : hs + 256]
        bqk[:, 2] = bk[hs : hs + 128]
        bqk[:, 3] = bk[hs + 128 : hs + 256]
        m = {
            "xq": arr_blocked(query[b].T, 8, NTB),
            "xk": arr_blocked(xk, 6, nkb),
            "xv": arr_blocked(xv, 6, nkb),
            "wq": arr_kmajor(Wq[:, hs : hs + GH], 8),
            "wk": arr_kmajor(Wk[:, hs : hs + GH], 6),
            "wv": arr_kmajor(Wv[:, hs : hs + GH], 6),
            "wo": arr_kmajor(Wo[hs : hs + GH, :], 2),
            "mbias": np.ascontiguousarray(mbias.reshape(nkt, 128).T),
            "bqk": bqk,
        }
        if with_bv:
            bvt = np.empty((128, 2), np.float32)
            bvt[:, 0] = bv[hs : hs + 128]
            bvt[:, 1] = bv[hs + 128 : hs + 256]
            m["bv"] = bvt
        in_maps.append(m)

    kwargs = {}
    if PROFILE:
        import tempfile

        LAST_TRACE_DIR = tempfile.mkdtemp(prefix="bass_trace_")
        kwargs = {"trace": True, "tmpdir": LAST_TRACE_DIR}
    res = run_bass_kernel_spmd(nc, in_maps, list(range(N_CORES)), **kwargs)
    LAST_EXEC_NS = res.exec_time_ns

    out = np.zeros((B, LQ, QDIM), np.float32)
    for c in range(N_CORES):
        blk = res.results[c]["outp"]  # [LQ//128, 2, 128, 512] block-major
        out[c // HG] += blk.transpose(0, 2, 1, 3).reshape(LQ, QDIM)
    out += bo[None, None, :]
    for b in range(B):
        if len(keep[b]) == 0:
            # all keys masked: reference softmax is NaN everywhere
            out[b] = np.nan
    # key_mask masks the QUERY axis in the reference; a zero row makes the
    # whole softmax row -inf -> NaN output for that query position.
    for b in range(B):
        zq = np.flatnonzero(km[b] == 0)
        if len(zq):
            out[b, zq, :] = np.nan
    return out



# revision 41
# speedup vs baseline: 1.1761x; 1.0117x over previous
"""Trainium2 Bass kernel for nn_CrossModalAttention (B=2, LQ=LK=2048,
QDIM=HID=1024, KDIM=VDIM=768, H=16, D=64).

Sharding: 8 cores = 2 batches x 4 head-groups (4 heads each).
Per core: q/k/v projections column-sliced over HID, attention for its 4
heads, row-parallel partial of the out-projection. Host sums the 4
partials per batch (the row-parallel unshard) and adds bo.

Device dataflow (per core), all matmuls in fp32r (TF32-like, ~1.5e-4):
  - host passes query/key/value[b] transposed (and K/V key-compacted:
    query_mask masks the KEY axis globally per batch, so masked keys are
    dropped on host and the remainder padded to a multiple of 128)
  - qT/kT [hid, tokens] and v [keys, hid] computed on device
  - per head pair (row-packed K=64 matmuls via tile_position):
    scoresT [keys, q] -> ACT exp(s/8 + mask_bias) -> PV matmul with a
    ones-augmented V (M=65) giving ctxT and the softmax denominator
  - normalize on DVE (reciprocal + gpsimd partition-broadcast)
  - out-projection from ctxT, partial written to DRAM
"""

import math

import ml_dtypes
import numpy as np

B, LQ, LK = 2, 2048, 2048
QDIM, KDIM, VDIM, HID, H = 1024, 768, 768, 1024, 16
D = HID // H  # 64
HG = 4  # head-groups (cores per batch)
HL = H // HG  # heads per core = 4
GH = HL * D  # per-core hid slice = 256
N_CORES = 8
TB = 512  # token block
NTB = LQ // TB  # 4
NEG = -1.0e30

BF16 = True
PROFILE = False
LAST_EXEC_NS = None
LAST_TRACE_DIR = None

_CACHE = {}


def _build(nkt: int, with_bv: bool, bf16: bool):
    import concourse.bacc as bacc
    import concourse.mybir as mybir
    import concourse.tile as tile

    nkeys = nkt * 128
    # key blocks of <=512 for the k-projection
    kbs = [min(512, nkeys - s) for s in range(0, nkeys, 512)]

    f32 = mybir.dt.float32
    f32r = mybir.dt.bfloat16 if bf16 else mybir.dt.float32r
    Exp = mybir.ActivationFunctionType.Exp
    Ident = mybir.ActivationFunctionType.Identity

    nc = bacc.Bacc(
        "TRN2", target_bir_lowering=False, debug=False, num_devices=N_CORES
    )

    nkb = len(kbs)
    # DRAM tensors (per-core shapes). Inputs are laid out block-major so
    # every DMA moves large contiguous per-partition lines (1KB strided
    # lines are descriptor-rate-bound: ~15us for 1MB)
    XQ = nc.dram_tensor(
        "xq", [128, NTB, 8, TB], f32r, kind="ExternalInput"
    ).ap()
    XK = nc.dram_tensor(
        "xk", [128, nkb, 6, 512], f32r, kind="ExternalInput"
    ).ap()
    XV = nc.dram_tensor(
        "xv", [128, nkb, 6, 512], f32r, kind="ExternalInput"
    ).ap()
    WQ = nc.dram_tensor("wq", [128, 8, GH], f32r, kind="ExternalInput").ap()
    WK = nc.dram_tensor("wk", [128, 6, GH], f32r, kind="ExternalInput").ap()
    WV = nc.dram_tensor("wv", [128, 6, GH], f32r, kind="ExternalInput").ap()
    WO = nc.dram_tensor("wo", [128, 2, QDIM], f32r, kind="ExternalInput").ap()
    MB = nc.dram_tensor("mbias", [128, nkt], f32, kind="ExternalInput").ap()
    BQ = nc.dram_tensor("bqk", [128, 4], f32, kind="ExternalInput").ap()
    BV = None
    if with_bv:
        BV = nc.dram_tensor("bv", [128, 2], f32, kind="ExternalInput").ap()
    # output block-major: [tt, nh, 128, 512] so each store is one fully
    # contiguous write; bf16 halves the drain (host sums partials in f32)
    OUT = nc.dram_tensor(
        "outp", [LQ // 128, 2, 128, 512], f32r, kind="ExternalOutput"
    ).ap()

    with tile.TileContext(nc) as tc:
        with (
            tc.tile_pool(name="consts", bufs=1) as consts,
            tc.tile_pool(name="resid", bufs=1) as resid,
            tc.tile_pool(name="xs", bufs=2) as xs,
            tc.tile_pool(name="probs", bufs=4) as probs_pool,
            tc.tile_pool(name="norm", bufs=3) as norm_pool,
            tc.tile_pool(name="outs", bufs=3) as outs_pool,
            tc.tile_pool(name="ps", bufs=2, space="PSUM") as ps,
        ):
            # ---- constants / weights ----
            # wk/wq gate the very first matmuls: they go FIRST on the sync
            # HWDGE ring (the gpsimd SWDGE ring takes ~7us to complete a
            # transfer, which used to stall the PE until ~15us). wv/wo are
            # needed later and stay on the SWDGE ring.
            wq_sb = consts.tile([128, 8, GH], f32r)
            wk_sb = consts.tile([128, 6, GH], f32r)
            wv_sb = consts.tile([128, 6, GH], f32r)
            wo_sb = consts.tile([128, 2, QDIM], f32r)
            mb_sb = consts.tile([128, nkt], f32)
            bqk_sb = consts.tile([128, 4], f32)
            nc.sync.dma_start(out=wk_sb, in_=WK)
            nc.scalar.dma_start(out=bqk_sb, in_=BQ)
            nc.scalar.dma_start(out=mb_sb, in_=MB)
            nc.gpsimd.dma_start(out=wv_sb, in_=WV)
            nc.gpsimd.dma_start(out=wo_sb, in_=WO)
            bv_sb = None
            if with_bv:
                bv_sb = consts.tile([128, 2], f32)
                nc.gpsimd.dma_start(out=bv_sb, in_=BV)

            # ---- residents ----
            # qT tiles double as ctxT tiles later (WAR handled by Tile)
            qT = [resid.tile([128, LQ], f32r, tag=f"qT{p}", name=f"qT{p}") for p in range(2)]
            kT = [resid.tile([128, nkeys], f32r, tag=f"kT{p}", name=f"kT{p}") for p in range(2)]
            xq_all = resid.tile([128, NTB, 8, TB], f32r, tag="xq", name="xq")
            v_sb = resid.tile([128, nkt, HL, D + 1], f32r)
            # ones columns for the denominator rows: fill the whole tile,
            # the v-projection copies then overwrite the [., ., ., 0:D] part
            if bf16:
                nc.vector.memset(v_sb, 1.0)
            else:
                nc.vector.memset(v_sb[:, :, :, :].bitcast(f32), 1.0)

            # ---- k projection (per key-block, so attention can begin
            # after kb0) ----
            def emit_xk(kb_i):
                kbw = kbs[kb_i]
                xk_t = xs.tile([128, 6, 512], f32r, tag="xk", name="xk_t", bufs=3)
                nc.sync.dma_start(
                    out=xk_t[:, :, :kbw], in_=XK[:, kb_i, :, :kbw]
                )
                return xk_t

            def emit_kproj_m(kb_i, m, xk_t):
                kbw = kbs[kb_i]
                s0 = kb_i * 512
                ps_t = ps.tile([128, 512], f32, tag="proj", name="kp_ps")
                for k in range(6):
                    nc.tensor.matmul(
                        ps_t[:, :kbw],
                        wk_sb[:, k, m * 128 : (m + 1) * 128],
                        xk_t[:, k, :kbw],
                        start=(k == 0),
                        stop=(k == 5),
                    )
                nc.vector.tensor_scalar_add(
                    kT[m][:, s0 : s0 + kbw],
                    ps_t[:, :kbw],
                    bqk_sb[:, 2 + m : 3 + m],
                )

            # ---- v projection (emitted later for tb0/p0 split; see
            # emit_vproj) ----
            vproj_state = {}

            def emit_xv(kb_i):
                if kb_i not in vproj_state:
                    kbw = kbs[kb_i]
                    xv_t = xs.tile(
                        [128, 6, 512], f32r, tag="xv", name="xv_t",
                        bufs=max(2, len(kbs)),
                    )
                    nc.sync.dma_start(
                        out=xv_t[:, :, :kbw], in_=XV[:, kb_i, :, :kbw]
                    )
                    vproj_state[kb_i] = xv_t
                return vproj_state[kb_i]

            def emit_vproj_kt(kt):
                kb_i = kt // 4
                sub = kt % 4
                xv_t = emit_xv(kb_i)
                ps_t = ps.tile([128, 512], f32, tag="proj", name="vp_ps")
                for k in range(6):
                    nc.tensor.matmul(
                        ps_t[:, :GH],
                        xv_t[:, k, sub * 128 : (sub + 1) * 128],
                        wv_sb[:, k, :],
                        start=(k == 0),
                        stop=(k == 5),
                    )
                nc.vector.tensor_copy(
                    v_sb[:, kt, :, 0:D],
                    ps_t[:, :GH].rearrange("p (h d) -> p h d", h=HL),
                )

            def emit_xq(tb, half=None):
                if half is None:
                    nc.sync.dma_start(out=xq_all[:, tb], in_=XQ[:, tb])
                else:
                    k0, k1 = (0, 4) if half == 0 else (4, 8)
                    nc.sync.dma_start(
                        out=xq_all[:, tb, k0:k1], in_=XQ[:, tb, k0:k1]
                    )

            def emit_qproj_m(tb, m):
                t0 = tb * TB
                ps_t = ps.tile([128, 512], f32, tag="proj", name="qp_ps")
                for k in range(8):
                    nc.tensor.matmul(
                        ps_t,
                        wq_sb[:, k, m * 128 : (m + 1) * 128],
                        xq_all[:, tb, k, :],
                        start=(k == 0),
                        stop=(k == 7),
                    )
                nc.vector.tensor_scalar_add(
                    qT[m][:, t0 : t0 + TB],
                    ps_t,
                    bqk_sb[:, m : m + 1],
                )

            def emit_scores(p, tb, kt, prtag, prbufs):
                t0 = tb * TB
                k0 = kt * 128
                sc = ps.tile([128, 2, TB], f32, tag="sc", name="sc")
                for hh in range(2):
                    nc.tensor.matmul(
                        sc[:, hh, :],
                        kT[p][hh * 64 : hh * 64 + 64, k0 : k0 + 128],
                        qT[p][hh * 64 : hh * 64 + 64, t0 : t0 + TB],
                        start=True,
                        stop=True,
                        tile_position=(hh * 64, 0),
                    )
                pr = probs_pool.tile(
                    [128, 2, TB], f32r, tag=prtag, name="pr", bufs=prbufs
                )
                nc.scalar.activation(
                    pr, sc, Exp, bias=mb_sb[:, kt : kt + 1], scale=0.125
                )
                return pr

            def emit_pv_hh(p, tb, kt, pr, ctx_ps, hh):
                nc.tensor.matmul(
                    ctx_ps[hh],
                    v_sb[:, kt, 2 * p + hh, :],
                    pr[:, hh, :],
                    start=(kt == 0),
                    stop=(kt == nkt - 1),
                )

            def emit_normalize(p, tb, ctx_ps):
                t0 = tb * TB
                # evacuate both PSUM ctx tiles to SBUF first (quick DVE
                # copies) so the next block's PV matmuls get their PSUM
                # slots immediately; the normalize chain then runs from
                # SBUF off the PE critical path
                evac = []
                for hh in range(2):
                    ctmp = norm_pool.tile([D, TB], f32, tag="ctmp", name="ctmp")
                    nc.vector.tensor_copy(ctmp, ctx_ps[hh][0:D, :])
                    dcp = norm_pool.tile([1, TB], f32, tag="dcp", name="dcp")
                    nc.vector.tensor_copy(dcp, ctx_ps[hh][D : D + 1, :])
                    evac.append((ctmp, dcp))
                for hh in range(2):
                    ctmp, dcp = evac[hh]
                    rbc = norm_pool.tile([D, TB], f32, tag="rbc", name="rbc")
                    nc.gpsimd.partition_broadcast(rbc, dcp)
                    rec = norm_pool.tile([D, TB], f32, tag="rec", name="rec")
                    nc.vector.reciprocal_approx_fast(out=rec, in_=rbc)
                    dst = qT[p][hh * 64 : hh * 64 + 64, t0 : t0 + TB]
                    nc.vector.tensor_mul(dst, ctmp, rec)
                    if with_bv:
                        nc.vector.tensor_scalar_add(
                            dst, dst, bv_sb[64 * hh : 64 * hh + 64, p : p + 1]
                        )

            def emit_attn2(p, tb, prs, thunks=()):
                # software-pipelined group: the probs for (p, tb) were
                # fully computed during the PREVIOUS group, so the PV
                # matmuls here never wait on an exp. The thunks (next
                # group's scores + a projection slice) provide the PE
                # stream and ACT feed for the NEXT group.
                thunks = list(thunks)
                ti = 0

                def run_thunk():
                    nonlocal ti
                    if ti < len(thunks):
                        thunks[ti]()
                        ti += 1

                ctx_ps = [
                    ps.tile([D + 1, TB], f32, tag="ctx", name=f"ctx{p}_{tb}_{i}")
                    for i in range(2)
                ]
                for kt in range(nkt):
                    run_thunk()
                    emit_pv_hh(p, tb, kt, prs[kt], ctx_ps, 0)
                    run_thunk()
                    emit_pv_hh(p, tb, kt, prs[kt], ctx_ps, 1)
                while ti < len(thunks):
                    thunks[ti]()
                    ti += 1
                emit_normalize(p, tb, ctx_ps)

            def qproj_thunks(tb):
                # 6 small thunks: 3+3+2 matmuls per m-half, chained into
                # one psum accumulation (interleaving with other banks is
                # legal), evac on the last
                cell = {}

                def chunk(m, k0, k1, fin, tb=tb):
                    if k0 == 0:
                        cell[m] = ps.tile(
                            [128, 512], f32, tag="proj", name="qp_ps"
                        )
                    ps_t = cell[m]
                    for k in range(k0, k1):
                        nc.tensor.matmul(
                            ps_t,
                            wq_sb[:, k, m * 128 : (m + 1) * 128],
                            xq_all[:, tb, k, :],
                            start=(k == 0),
                            stop=(k == 7),
                        )
                    if fin:
                        nc.vector.tensor_scalar_add(
                            qT[m][:, tb * TB : (tb + 1) * TB],
                            ps_t,
                            bqk_sb[:, m : m + 1],
                        )

                out = []
                for m in range(2):
                    out += [
                        lambda m=m: chunk(m, 0, 3, False),
                        lambda m=m: chunk(m, 3, 6, False),
                        lambda m=m: chunk(m, 6, 8, True),
                    ]
                return out

            def outproj_pieces(tb):
                out = []
                for tt in range(4 * tb, 4 * tb + 4):
                    for nh in range(2):
                        out.append(
                            lambda tt=tt, nh=nh: emit_outproj_nh(tt, nh)
                        )
                return out

            def emit_outproj_nh(tt, nh):
                # evac on DVE (keeps ACT free for the exps) and DMA on the
                # vector HWDGE ring so the sync ring stays dedicated to the
                # big input streams
                ps_t = ps.tile([128, 512], f32, tag="proj", name="op_ps")
                for kk in range(2):
                    nc.tensor.matmul(
                        ps_t,
                        qT[kk][:, tt * 128 : (tt + 1) * 128],
                        wo_sb[:, kk, nh * 512 : (nh + 1) * 512],
                        start=(kk == 0),
                        stop=(kk == 1),
                    )
                o_sb = outs_pool.tile(
                    [128, 512], f32r, tag="osb", name="o_sb"
                )
                nc.vector.tensor_copy(o_sb, ps_t)
                nc.sync.dma_start(out=OUT[tt, nh], in_=o_sb)

            def emit_outproj(tb):
                for tt in range(4 * tb, 4 * tb + 4):
                    for nh in range(2):
                        emit_outproj_nh(tt, nh)

            # ---- emission schedule ----
            # DMA priority order on the sync ring: xk0, xq0, xk1.., xv
            # blocks, then the remaining xq tiles — everything is issued
            # up front so out-DMAs appended later never delay inputs.
            # Preamble: minimal PE path to the first exp (kb0/m0 + q/m0 +
            # scores kt0..3); steady state: every attention group carries
            # PE filler thunks (qproj of tb+1 or outproj of tb-1) so the
            # PE never runs a matmul-only phase while ACT idles
            xk_ts = [emit_xk(0)]
            nc.sync.dma_start(out=wq_sb, in_=WQ)
            emit_xq(0, half=0)
            emit_xq(0, half=1)
            for kb_i in range(1, len(kbs)):
                xk_ts.append(emit_xk(kb_i))
            for kb_i in range(len(kbs)):
                emit_xv(kb_i)
            for tb in range(1, NTB):
                emit_xq(tb)
            # emission in DMA-readiness order: both kproj halves only need
            # wk+xk0 (the first arrivals) and fill the PE while wq/xq0
            # stream in
            emit_kproj_m(0, 0, xk_ts[0])
            emit_kproj_m(0, 1, xk_ts[0])
            emit_qproj_m(0, 0)
            kt_kb0 = min(4, nkt)
            prs0 = [emit_scores(0, 0, kt, "pr0", nkt) for kt in range(kt_kb0)]
            emit_qproj_m(0, 1)
            for kb_i in range(1, len(kbs)):
                emit_kproj_m(kb_i, 0, xk_ts[kb_i])
                emit_kproj_m(kb_i, 1, xk_ts[kb_i])
            prs0 += [
                emit_scores(0, 0, kt, "pr0", nkt) for kt in range(kt_kb0, nkt)
            ]
            for kt in range(nkt):
                emit_vproj_kt(kt)

            # one-group software pipeline: group gi's PV pass carries the
            # scores of group gi+1 (alternating pr0/pr1 generations) plus
            # a projection slice as thunks. qproj(tb+1) rides the (0, tb)
            # group; outproj(tb-1) rides (1, tb) — a full group after the
            # normalize it consumes, so it never stalls the PE queue.
            order = [(p_, tb_) for tb_ in range(NTB) for p_ in (0, 1)]
            prs_map = {0: prs0}

            def mk_scores_thunks(gi):
                p_, tb_ = order[gi]
                store = prs_map.setdefault(gi, [])
                tag = "pr0" if gi % 2 == 0 else "pr1"
                return [
                    (lambda kt=kt: store.append(
                        emit_scores(p_, tb_, kt, tag, nkt)
                    ))
                    for kt in range(nkt)
                ]

            def interleave(a, b):
                out = []
                for i in range(max(len(a), len(b))):
                    if i < len(a):
                        out.append(a[i])
                    if i < len(b):
                        out.append(b[i])
                return out

            def sched_thunks(sth, proj, offset=0):
                # scores thunks take every slot; proj thunks join in from
                # slot `offset` (delaying past a not-yet-finished normalize)
                out = []
                si = pi = slot = 0
                while si < len(sth) or pi < len(proj):
                    if si < len(sth):
                        out.append(sth[si])
                        si += 1
                    if slot >= offset and pi < len(proj):
                        out.append(proj[pi])
                        pi += 1
                    slot += 1
                return out

            # projection slices ride the earliest group where their
            # dependencies are done: qproj(tb+1) on groups 0..2;
            # outproj(tb) split across the two groups of tb+2 (offset-4
            # in the (0, ·) group because normalize(1, tb+1) only lands
            # ~2us into it); outproj(NTB-1) drains at the tail
            proj_for = {
                (0, 0): (qproj_thunks(1), 0),
                (1, 0): (qproj_thunks(2) if NTB > 2 else [], 0),
                (0, 1): (qproj_thunks(3) if NTB > 3 else [], 0),
                (1, 1): (outproj_pieces(0), 0),
                (0, 2): (outproj_pieces(1)[:4], 4),
                (1, 2): (outproj_pieces(1)[4:], 0),
                (0, 3): (outproj_pieces(2)[:4], 4),
                (1, 3): (outproj_pieces(2)[4:], 0),
            }
            for gi, (p_, tb_) in enumerate(order):
                sth = mk_scores_thunks(gi + 1) if gi + 1 < len(order) else []
                proj, off = proj_for.get((p_, tb_), ([], 0))
                emit_attn2(p_, tb_, prs_map[gi], sched_thunks(sth, proj, off))
            emit_outproj(NTB - 1)

    nc.compile()
    return nc


def kernel(
    query, key, value, Wq, bq, Wk, bk, Wv, bv, Wo, bo, query_mask, key_mask
):
    global LAST_EXEC_NS, LAST_TRACE_DIR
    from concourse.bass_utils import run_bass_kernel_spmd

    query = np.asarray(query, dtype=np.float32)
    key = np.asarray(key, dtype=np.float32)
    value = np.asarray(value, dtype=np.float32)
    Wq = np.asarray(Wq, dtype=np.float32)
    Wk = np.asarray(Wk, dtype=np.float32)
    Wv = np.asarray(Wv, dtype=np.float32)
    Wo = np.asarray(Wo, dtype=np.float32)
    bq = np.asarray(bq, dtype=np.float32)
    bk = np.asarray(bk, dtype=np.float32)
    bv = np.asarray(bv, dtype=np.float32)
    bo = np.asarray(bo, dtype=np.float32)
    qm = np.asarray(query_mask)
    km = np.asarray(key_mask)

    # host-side key compaction (query_mask masks the KEY axis, globally
    # per batch)
    keep = [np.flatnonzero(qm[b] != 0) for b in range(B)]
    nkeep = max((len(k) for k in keep), default=0)
    nkt = max(1, math.ceil(nkeep / 128))
    nkeys = nkt * 128

    with_bv = bool(np.any(bv))
    ck = (nkt, with_bv, BF16)
    if ck not in _CACHE:
        _CACHE[ck] = _build(nkt, with_bv, BF16)
    nc = _CACHE[ck]

    wdt = ml_dtypes.bfloat16 if BF16 else np.float32
    nkb = math.ceil(nkeys / 512)
    nkeys_b = nkb * 512

    def arr_kmajor(a, ktiles):  # [dim, n] -> [128, ktiles, n]
        return np.ascontiguousarray(
            a.reshape(ktiles, 128, a.shape[1]).transpose(1, 0, 2)
        ).astype(wdt)

    def arr_blocked(a, ktiles, blocks):  # [dim, n] -> [128, blocks, kt, 512]
        return np.ascontiguousarray(
            a.reshape(ktiles, 128, blocks, 512).transpose(1, 2, 0, 3)
        ).astype(wdt)

    in_maps = []
    for c in range(N_CORES):
        b, hg = c // HG, c % HG
        hs = hg * GH
        idx = keep[b]
        # compacted + padded key/value (transposed)
        xk = np.zeros((KDIM, nkeys_b), np.float32)
        xk[:, : len(idx)] = key[b].T[:, idx]
        xv = np.zeros((VDIM, nkeys_b), np.float32)
        xv[:, : len(idx)] = value[b].T[:, idx]
        mbias = np.full((nkeys,), NEG, np.float32)
        mbias[: len(idx)] = 0.0
        bqk = np.empty((128, 4), np.float32)
        bqk[:, 0] = bq[hs : hs + 128]
        bqk[:, 1] = bq[hs + 128 : hs + 256]
        bqk[:, 2] = bk[hs : hs + 128]
        bqk[:, 3] = bk[hs + 128 : hs + 256]
        m = {
            "xq": arr_blocked(query[b].T, 8, NTB),
            "xk": arr_blocked(xk, 6, nkb),
            "xv": arr_blocked(xv, 6, nkb),
            "wq": arr_kmajor(Wq[:, hs : hs + GH], 8),
            "wk": arr_kmajor(Wk[:, hs : hs + GH], 6),
            "wv": arr_kmajor(Wv[:, hs : hs + GH], 6),
            "wo": arr_kmajor(Wo[hs : hs + GH, :], 2),
            "mbias": np.ascontiguousarray(mbias.reshape(nkt, 128).T),
            "bqk": bqk,
        }
        if with_bv:
            bvt = np.empty((128, 2), np.float32)
            bvt[:, 0] = bv[hs : hs + 128]
            bvt[:, 1] = bv[hs + 128 : hs + 256]
            m["bv"] = bvt
        in_maps.append(m)

    kwargs = {}
    if PROFILE:
        import tempfile

        LAST_TRACE_DIR = tempfile.mkdtemp(prefix="bass_trace_")
        kwargs = {"trace": True, "tmpdir": LAST_TRACE_DIR}
    res = run_bass_kernel_spmd(nc, in_maps, list(range(N_CORES)), **kwargs)
    LAST_EXEC_NS = res.exec_time_ns

    out = np.zeros((B, LQ, QDIM), np.float32)
    for c in range(N_CORES):
        blk = res.results[c]["outp"]  # [LQ//128, 2, 128, 512] block-major
        out[c // HG] += blk.transpose(0, 2, 1, 3).reshape(LQ, QDIM)
    out += bo[None, None, :]
    for b in range(B):
        if len(keep[b]) == 0:
            # all keys masked: reference softmax is NaN everywhere
            out[b] = np.nan
    # key_mask masks the QUERY axis in the reference; a zero row makes the
    # whole softmax row -inf -> NaN output for that query position.
    for b in range(B):
        zq = np.flatnonzero(km[b] == 0)
        if len(zq):
            out[b, zq, :] = np.nan
    return out

